# revision 12
# baseline (speedup 1.0000x reference)
"""CodecAttention (sliding-window attention w/ QK-RMSNorm + ALiBi) on 8 trn2 cores.

Sharding: data-parallel over (batch, sequence-chunk): 2 batches x 4 chunks of 512
queries -> 8 cores. Each core recomputes K/V for its 512-token halo (zero-padded
for the first chunk), so there is no cross-core communication; the host only
slices/transposes inputs and concatenates the 8 disjoint output slices.

On-core pipeline (bf16 operands, fp32 PSUM accumulation):
  A) QKV projections from x^T (dim-on-partitions); QK RMSNorm over the flat
     1024-dim axis via ACT-square + accumulating ones-matmul partition
     reduction, rsqrt as exp(-0.5*ln(x)), 1/sqrt(dh) folded into the q scale,
     q/k gamma folded into wq/wk on the host.
  B) Head pairs: S^T tiles = k^T.T @ q^T with the ALiBi bias folded into the
     matmul as a concurrent row-tiled K=4 matmul (hi/lo bf16-split position
     rows; padded halo keys of chunk 0 killed via a -3e4 key bias), ACT exp
     with a fixed offset straight off PSUM (scores bounded), gpsimd
     affine_select/memset zero the causal/window boundary triangles and
     dead rectangles on the post-exp pt tiles, AV+rowsum with V-as-stationary
     (ones column yields softmax denominators), denominators DMA'd from PSUM
     into an [8, TQ] tile, one grouped reciprocal, K=8 selection-matrix
     matmul broadcasts the reciprocals per pair, per-pair normalize multiply.
  C) out = attnT.T @ wo^T per token tile, streamed 512KB output DMAs.
"""

import contextlib
import ctypes
import os
import sys
import types

import ml_dtypes
import numpy as np

import concourse.bass as bass
import concourse.mybir as mybir
import concourse.tile as tile


def _install_axon_ntff_shim():
    """bass_utils' trace path wants antenv.axon_hooks, which this image lacks.
    Provide it, backed by direct ctypes calls into libaxon_pjrt.so (same ABI
    the agent boot would use). Degrades to hook=None if the .so is absent."""
    try:
        import antenv.axon_hooks  # noqa: F401
        return
    except ImportError:
        pass

    _hook_holder = [None]
    so_path = "/opt/axon/libaxon_pjrt.so"
    if os.path.exists(so_path):
        try:
            lib = ctypes.CDLL(so_path)
            if hasattr(lib, "axon_start_nrt_profile"):
                lib.axon_start_nrt_profile.argtypes = [
                    ctypes.POINTER(ctypes.c_int64), ctypes.c_size_t]
                lib.axon_start_nrt_profile.restype = ctypes.c_int64
                lib.axon_stop_nrt_profile.argtypes = [ctypes.c_char_p]
                lib.axon_stop_nrt_profile.restype = ctypes.c_int64

                @contextlib.contextmanager
                def _hook(output_dir, device_ids):
                    import jax
                    jax.devices()
                    if device_ids:
                        ids = (ctypes.c_int64 * len(device_ids))(*device_ids)
                        rc = lib.axon_start_nrt_profile(ids, len(device_ids))
                    else:
                        rc = lib.axon_start_nrt_profile(None, 0)
                    if rc != 0:
                        raise RuntimeError(f"axon_start_nrt_profile rc={rc}")
                    try:
                        yield
                    finally:
                        n = lib.axon_stop_nrt_profile(str(output_dir).encode())
                        if n < 0:
                            raise RuntimeError(f"axon_stop_nrt_profile rc={n}")

                _hook_holder[0] = _hook
        except OSError:
            pass

    mod = types.ModuleType("antenv.axon_hooks")
    mod.get_axon_ntff_profile_hook = lambda: _hook_holder[0]
    mod.set_axon_ntff_profile_hook = lambda h: _hook_holder.__setitem__(0, h)
    sys.modules["antenv.axon_hooks"] = mod


_install_axon_ntff_shim()

from concourse.bass_utils import run_bass_kernel_spmd  # noqa: E402
from bass_rust import ScopedClock  # noqa: E402

B, T, DIM = 2, 2048, 1024
H, DH, WINDOW = 16, 64, 512
P = 128
TQ = 512            # queries per core
TKV = 1024          # kv tokens per core (incl. 512 halo)
NCORES = 8
NQT = TQ // P       # 4
NKT = TKV // P      # 8
NDC = DIM // P      # 8
EXP_C = 10.0        # exp offset; true max masked score is ~6.0 for this data
F32 = mybir.dt.float32
F32R = mybir.dt.float32r
BF16 = mybir.dt.bfloat16
PREC = os.environ.get("KERNEL_PREC", "bf16")
DT = F32R if PREC == "fp32r" else BF16
AF = mybir.ActivationFunctionType
ALU = mybir.AluOpType

SLOPES = [2.0 ** (-0.5 * (h + 1)) for h in range(H)]

# Score-tile packing: per head, scores are computed as S^T [key, query] in two
# 3-bank PSUM halves of [128, 1536]. Key-tile kt covers queries
# [KT_QLO[kt], KT_QLO[kt]+KT_W[kt]) at column offset KT_OFF[kt] of its half.
KT_W = [256, 256, 512, 512, 512, 512, 256, 256]
KT_OFF = [0, 256, 512, 1024, 0, 512, 1024, 1280]
KT_QLO = [0, 0, 0, 0, 0, 0, 256, 256]
HW_HALF = 1536
# post-exp boundary cleanup per half: (col0, kind) with kind 'win' (keep
# key>=query within the block), 'causal' (keep query>=key), 'dead' (zero all).
# Block (kt, qb) is valid iff qb <= kt <= qb+4; kt==qb is the window edge,
# kt==qb+4 the causal edge; anything else computed by the packing is dead.
PT_FIX = {
    0: [(0, "win"), (128, "dead"), (256 + 128, "win"),
        (512 + 256, "win"), (512 + 384, "dead"), (1024 + 384, "win")],
    1: [(0, "causal"), (512, "dead"), (512 + 128, "causal"),
        (1024, "causal"), (1280, "dead"), (1280 + 128, "causal")],
}
# 256-query output blocks: which key tiles contribute to each
QB_KTS = {0: [0, 1, 2, 3, 4, 5], 1: [2, 3, 4, 5, 6, 7]}
# merged AV matmul plan: (kt, out_col_lo, width, start, stop)
AV_PLAN = [
    (0, 0, 256, True, False),
    (1, 0, 256, False, False),
    (2, 0, 256, False, False), (2, 256, 256, True, False),
    (3, 0, 512, False, False),
    (4, 0, 512, False, False),
    (5, 0, 512, False, False),
    (6, 256, 256, False, False),
    (7, 256, 256, False, True),
]


class _SplitDrainTileContext(tile.TileContext):
    """The walrus build in this env rejects >1-2 sync-wait commands on one
    instruction; spread excess waits across same-engine NOPs placed directly
    before the over-limit instruction (per-engine program order preserved)."""

    def _split_excess_waits(self):
        nc = self.nc
        cur_list = nc.cur_bb.bb.instructions
        for blk in nc.m.functions[0].blocks:
            snapshot = list(blk.instructions)
            for inst in snapshot:
                si = inst.sync_info
                max_w = 1
                if si is None or len(si.on_wait) <= max_w:
                    continue
                waits = list(si.on_wait)
                si.on_wait = waits[:max_w]
                eng_obj = nc.engines[inst.engine]
                for w in waits[max_w:]:
                    nop_bi = eng_obj.nop(nofuse=True, hint="wait_split")
                    nop_inst = nop_bi.ins
                    nop_inst.sync_info = mybir.SyncInfo(on_wait=[w], on_update=[])
                    cur_list.remove(nop_inst)
                    blk.instructions.insert(
                        blk.instructions.index(inst), nop_inst)

    def _drain_and_barrier(self, tick_clock, wait_clock):
        self._split_excess_waits()
        drain_inst = self.nc.sync.drain()
        wait_clock.add_sem_waits(
            drain_inst.ins, ScopedClock({None: tick_clock.global_clock})
        )
        si = drain_inst.ins.sync_info
        if si is not None and len(si.on_wait) > 1:
            waits = list(si.on_wait)
            si.on_wait = waits[:1]
            for w in waits[1:]:
                nop = self.nc.sync.nop(nofuse=True, hint="drain_wait_split")
                nop.ins.sync_info = mybir.SyncInfo(on_wait=[w], on_update=[])
        self.nc.all_engine_barrier()
        assert self.sems is not None
        popped = self.nc._tile_sem_poison_stack.pop()
        assert popped is self._sem_poison
        self.nc.clear_and_free_semaphores(list(self.sems.allocated().values()))
        self.nc.all_engine_barrier()


def _src_nonce():
    import zlib
    with open(__file__, "rb") as f:
        return (zlib.crc32(f.read() + PREC.encode()) % 2048) + 8


def _pt_fixups(nc, pt, half):
    """Zero the invalid regions of a post-exp pt tile on the gpsimd engine."""
    for col0, kind in PT_FIX[half]:
        sl = pt[:, col0:col0 + P]
        if kind == "dead":
            nc.gpsimd.memset(sl, 0.0)
        elif kind == "win":
            # keep iff key_local >= query_local  (iota = p - qi >= 0)
            nc.gpsimd.affine_select(
                out=sl, in_=sl, compare_op=ALU.is_ge, fill=0.0,
                base=0, pattern=[[-1, P]], channel_multiplier=1)
        else:
            # causal: keep iff query_local >= key_local (iota = qi - p >= 0)
            nc.gpsimd.affine_select(
                out=sl, in_=sl, compare_op=ALU.is_ge, fill=0.0,
                base=0, pattern=[[1, P]], channel_multiplier=-1)


def _build_program(debug=False):
    nc = bass.Bass()
    # dummy input whose shape changes with this file: busts HLO-keyed NEFF
    # caches (the BIR itself is not part of the HLO fingerprint)
    nonce = nc.declare_dram_parameter("nonce", [1, _src_nonce()], F32,
                                      isOutput=False)
    xT = nc.declare_dram_parameter("xT", [DIM, TKV], DT, isOutput=False)
    wT = nc.declare_dram_parameter("wT", [DIM, 3 * DIM], DT, isOutput=False)
    woT = nc.declare_dram_parameter("woT", [DIM, DIM], DT, isOutput=False)
    qext = nc.declare_dram_parameter("qext", [4, H, TQ], DT, isOutput=False)
    kext = nc.declare_dram_parameter("kext", [4, H, TKV], DT, isOutput=False)
    gam = nc.declare_dram_parameter("gam", [P, 2 * NDC], F32, isOutput=False)
    out = nc.declare_dram_parameter("out", [TQ, DIM], F32, isOutput=True)
    if debug:
        qT_d = nc.declare_dram_parameter("qT_d", [P, NDC, TQ], DT, isOutput=True)
        kT_d = nc.declare_dram_parameter("kT_d", [P, NDC, TKV], DT, isOutput=True)
        V_d = nc.declare_dram_parameter("V_d", [P, NKT, H, DH + 1], DT, isOutput=True)
        aT_d = nc.declare_dram_parameter("aT_d", [P, NDC, TQ], DT, isOutput=True)
        pt_d = nc.declare_dram_parameter("pt_d", [2, P, 2, HW_HALF], DT, isOutput=True)

    with _SplitDrainTileContext(nc) as tc, \
            tc.tile_pool(name="persist", bufs=1) as pp, \
            tc.tile_pool(name="small", bufs=1) as psm:

        qT = pp.tile([P, NDC, TQ], DT, tag="qT")       # [p, odt, tok]
        kT = pp.tile([P, NDC, TKV], DT, tag="kT")
        V = pp.tile([P, NKT, H, DH + 1], DT, tag="V")  # [p=tok, kt, h, dh+ones]
        attnT = pp.tile([P, NDC, TQ], DT, tag="attnT")
        qext_sb = pp.tile([P, H, TQ], DT, tag="qext")
        kext_sb = pp.tile([P, H, TKV], DT, tag="kext")
        gam_sb = pp.tile([P, 2 * NDC], F32, tag="gam")
        nc.sync.dma_start(gam_sb[:], gam[:])
        ones_sb = pp.tile([P, 1], F32R, tag="ones")
        ones_row = pp.tile([1, P], F32, tag="onesrow")
        negc_sb = pp.tile([P, 1], F32, tag="negc")
        eps_sb = pp.tile([1, 1], F32, tag="eps")
        ln8_sb = pp.tile([1, 1], F32, tag="ln8")
        # per-pair selection matrices for the reciprocal broadcast matmul:
        # sel[j][g, p] = 1 iff g == 2j + p//64   (K=8 stationary, bf16)
        sel_sb = pp.tile([8, 4, P], BF16, tag="sel")
        nc.vector.memset(ones_sb[:].bitcast(F32), 1.0)
        nc.vector.memset(ones_row[:], 1.0)
        nc.vector.memset(negc_sb[:], -EXP_C)
        nc.vector.memset(eps_sb[:], 1.0e-6)
        nc.vector.memset(ln8_sb[:], float(-0.5 * np.log(64.0)))
        nc.gpsimd.memset(sel_sb[:], 0.0)
        for j in range(4):
            nc.gpsimd.affine_select(
                out=sel_sb[:, j, :], in_=sel_sb[:, j, :],
                compare_op=ALU.not_equal, fill=1.0,
                base=-2 * j, pattern=[[-1, 2], [0, 64]], channel_multiplier=1)
        nonce_sb = pp.tile([1, _src_nonce()], F32, tag="nonce")
        nc.sync.dma_start(nonce_sb[:], nonce[:])
        # alibi rows, replicated at partition bases 0 and 64 so the K=4
        # side-matmul never shares PE row-groups with the main K=64 matmul
        nc.sync.dma_start(qext_sb[64:68], qext[:])
        nc.sync.dma_start(qext_sb[0:4], qext[:])
        nc.sync.dma_start(kext_sb[64:68], kext[:])
        nc.sync.dma_start(kext_sb[0:4], kext[:])
        ones_col = V[:, :, :, DH]
        nc.vector.memset(
            ones_col if DT == BF16 else ones_col.bitcast(F32), 1.0)

        # ---------------- Phase A: projections + RMSNorm ----------------
        with tc.tile_pool(name="xp", bufs=1) as px, \
                tc.tile_pool(name="wp", bufs=2) as pw, \
                tc.tile_pool(name="sqp", bufs=2) as psq, \
                tc.tile_pool(name="accp", bufs=1) as pacc, \
                tc.tile_pool(name="psA", bufs=4, space="PSUM") as psA, \
                tc.tile_pool(name="psS1", bufs=2, space="PSUM") as psS1, \
                tc.tile_pool(name="psBC", bufs=2, space="PSUM") as psBC:

            x_sb = px.tile([P, NDC, TKV], DT, tag="x")
            wq0_sb = pw.tile([P, NDC, 512], DT, tag="wslice", name="wq0")
            wq0_src = wT[:, 0:512].rearrange("(dc p) o -> p dc o", p=P)
            # priority order: x tokens 512.. (Q + K-group-2 operands) first
            for dc in range(NDC):
                nc.sync.dma_start(x_sb[:, dc, 512:1024],
                                  xT[dc * P:(dc + 1) * P, 512:1024])
            for dc in range(NDC):
                nc.sync.dma_start(wq0_sb[:, dc, :], wq0_src[:, dc, :])
                nc.sync.dma_start(x_sb[:, dc, 0:512],
                                  xT[dc * P:(dc + 1) * P, 0:512])

            # Q (tokens 512..1023 of the kv range) and K (all tokens);
            # K group (512,512) first so it only needs the priority x half
            sqacc = {}
            for proj in range(2):
                dst = qT if proj == 0 else kT
                groups = [(TKV - TQ, 0)] if proj == 0 else [(512, 512), (0, 0)]
                for wh in range(2):
                    if proj == 0 and wh == 0:
                        w_sb = wq0_sb
                    else:
                        w_sb = pw.tile([P, NDC, 512], DT, tag="wslice")
                        nc.sync.dma_start(
                            w_sb[:],
                            wT[:, proj * DIM + wh * 512: proj * DIM + (wh + 1) * 512]
                            .rearrange("(dc p) o -> p dc o", p=P),
                        )
                    for ol in range(4):
                        odt = wh * 4 + ol
                        for (soff, doff) in groups:
                            ps = psA.tile([P, 512], F32, tag="projps")
                            for dc in range(NDC):
                                nc.tensor.matmul(
                                    ps[:],
                                    w_sb[:, dc, ol * P:(ol + 1) * P],
                                    x_sb[:, dc, soff:soff + 512],
                                    start=(dc == 0), stop=(dc == NDC - 1),
                                )
                            nc.scalar.copy(dst[:, odt, doff:doff + 512], ps[:])
                            sq = psq.tile([P, 512], F32, tag="sq")
                            nc.scalar.activation(sq[:], ps[:], AF.Square)
                            key = (proj, doff)
                            if odt == 0:
                                acc = pacc.tile([P, 512], F32,
                                                tag=f"acc{proj}_{doff}",
                                                name="acc")
                                sqacc[key] = acc
                                nc.vector.tensor_copy(acc[:], sq[:])
                            else:
                                nc.vector.tensor_add(sqacc[key][:],
                                                     sqacc[key][:], sq[:])

            # V projection: [tok, head, dh]; vh innermost so consecutive
            # matmuls share the x-chunk stationary operand (ldw-opt dedups);
            # token tiles 4..7 first (their x half arrives first)
            wv_sb = []
            for vh in range(2):
                w_sb = pw.tile([P, NDC, 512], DT, tag="wslice")
                nc.sync.dma_start(
                    w_sb[:],
                    wT[:, 2 * DIM + vh * 512: 2 * DIM + (vh + 1) * 512]
                    .rearrange("(dc p) o -> p dc o", p=P),
                )
                wv_sb.append(w_sb)
            for tt in (4, 5, 6, 7, 0, 1, 2, 3):
                pss = [psA.tile([P, 512], F32, tag="projps", name="psv")
                       for _ in range(2)]
                for dc in range(NDC):
                    for vh in range(2):
                        nc.tensor.matmul(
                            pss[vh][:],
                            x_sb[:, dc, tt * P:(tt + 1) * P],
                            wv_sb[vh][:, dc, :],
                            start=(dc == 0), stop=(dc == NDC - 1),
                        )
                for vh in range(2):
                    nc.scalar.copy(
                        V[:, tt, vh * 8:(vh + 1) * 8, :DH],
                        pss[vh][:].rearrange("p (h c) -> p h c", c=DH),
                    )

            # rsqrt(mean+eps) = exp(-0.5*ln(ss/DIM + eps)); the 1/sqrt(dh)
            # score scale folds into the exp bias for q
            bcasts = {}
            for (proj, doff), acc in sqacc.items():
                ss = psS1.tile([1, 512], F32, tag="ssq", name="ssq")
                nc.tensor.matmul(ss[:], ones_sb[:].bitcast(F32), acc[:],
                                 start=True, stop=True)
                a = psm.tile([1, 512], F32, tag="a")
                nc.scalar.activation(a[:], ss[:], AF.Ln,
                                     bias=eps_sb[:], scale=1.0 / DIM)
                y = psm.tile([1, 512], F32, tag="y")
                nc.scalar.activation(y[:], a[:], AF.Exp,
                                     bias=(ln8_sb[:] if proj == 0 else 0.0),
                                     scale=-0.5)
                # broadcast over partitions via K=1 ones-matmul (plain fp32)
                bc = psBC.tile([P, 512], F32, tag="bc", name="bc")
                nc.tensor.matmul(bc[:], ones_row[:], y[:], start=True, stop=True)
                bcasts[(proj, doff)] = bc

            # normalize in place (x gamma); odt-major order so head pair 0
            # becomes ready first
            for odt in range(NDC):
                for proj in range(2):
                    dst = qT if proj == 0 else kT
                    gap = gam_sb[:, proj * NDC + odt: proj * NDC + odt + 1]
                    for doff in ([0] if proj == 0 else [0, 512]):
                        sl = dst[:, odt, doff:doff + 512]
                        nc.vector.scalar_tensor_tensor(
                            sl, sl, gap, bcasts[(proj, doff)][:],
                            op0=ALU.mult, op1=ALU.mult,
                        )

        if debug:
            nc.sync.dma_start(qT_d[:], qT[:])
            nc.sync.dma_start(kT_d[:], kT[:])
            nc.sync.dma_start(V_d[:], V[:])

        # ---------------- Phase B: attention (head pairs) ----------------
        with tc.tile_pool(name="maskp", bufs=1) as pm, \
                tc.tile_pool(name="ptp", bufs=2) as ppt, \
                tc.tile_pool(name="rnp", bufs=1) as prn, \
                tc.tile_pool(name="psS", bufs=1, space="PSUM") as psS, \
                tc.tile_pool(name="psO", bufs=1, space="PSUM") as psO:

            s16 = [pm.tile([8, TQ], F32, tag="s16a", name="s16a"),
                   pm.tile([8, TQ], F32, tag="s16b", name="s16b")]

            for hp in range(NDC):
                ps_o = [psO.tile([DH + 1, TQ], F32, tag=f"avps{hi}", name=f"avps{hi}")
                        for hi in range(2)]
                pts = {0: [], 1: []}
                for half in range(2):
                    ps_pair = [psS.tile([P, HW_HALF], F32, tag=f"sps{hi}", name=f"sps{hi}")
                               for hi in range(2)]
                    for ktl in range(4):
                        kt = half * 4 + ktl
                        off, wdt, qlo = KT_OFF[kt], KT_W[kt], KT_QLO[kt]
                        for hi in range(2):
                            po = DH * hi
                            ep = DH * (1 - hi)
                            h = 2 * hp + hi
                            # ALiBi rank-2 rows: issued first (start=True) on
                            # the opposite PE row-group so it runs concurrently
                            # with the main K=64 matmul and drains earlier
                            nc.tensor.matmul(
                                ps_pair[hi][:, off:off + wdt],
                                kext_sb[ep:ep + 4, h, kt * P:(kt + 1) * P],
                                qext_sb[ep:ep + 4, h, qlo:qlo + wdt],
                                start=True, stop=False,
                                tile_position=(ep, 0),
                            )
                            nc.tensor.matmul(
                                ps_pair[hi][:, off:off + wdt],
                                kT[po:po + DH, hp, kt * P:(kt + 1) * P],
                                qT[po:po + DH, hp, qlo:qlo + wdt],
                                start=False, stop=True,
                                tile_position=(po, 0),
                            )
                    for hi in range(2):
                        pt = ppt.tile([P, HW_HALF], DT, tag=f"pt{hi}")
                        nc.scalar.activation(pt[:], ps_pair[hi][:], AF.Exp,
                                             bias=negc_sb[:])
                        _pt_fixups(nc, pt, half)
                        if debug and hp == 0:
                            nc.sync.dma_start(pt_d[hi, :, half, :], pt[:])
                        pts[hi].append(pt)

                for hi in range(2):
                    h = 2 * hp + hi
                    po = DH * hi
                    for qb in range(2):
                        kts = QB_KTS[qb]
                        for i, kt in enumerate(kts):
                            half, off, qlo = kt // 4, KT_OFF[kt], KT_QLO[kt]
                            c0 = off + qb * 256 - qlo
                            nc.tensor.matmul(
                                ps_o[hi][:, qb * 256:(qb + 1) * 256],
                                V[:, kt, h, :],
                                pts[hi][half][:, c0:c0 + 256],
                                start=(i == 0), stop=(i == len(kts) - 1),
                            )
                    # stash the softmax denominator row; head h -> partition
                    # h of s16 via a tiny SBUF->SBUF DMA (engines cannot write
                    # partition offsets other than 0/32/64)
                    r = prn.tile([1, TQ], F32, tag=f"r{hi}", name="r")
                    nc.vector.tensor_copy(r[:], ps_o[hi][DH:DH + 1, :])
                    nc.sync.dma_start(s16[h // 8][h % 8:h % 8 + 1, :], r[:])
                    nc.vector.tensor_copy(attnT[po:po + DH, hp, :], ps_o[hi][:DH, :])

                if hp % 4 == 3:
                    # normalize the finished half (8 heads): batched reciprocal
                    # on 8 partitions, K=8 selection-matrix matmul broadcasts
                    # each pair's rows across its 128 partitions
                    g = hp // 4
                    rc8 = pm.tile([8, TQ], F32, tag=f"rc{g}", name="rc8")
                    nc.vector.reciprocal(rc8[:], s16[g][:])
                    rc8b = pm.tile([8, TQ], BF16, tag=f"rcb{g}", name="rc8b")
                    nc.vector.tensor_copy(rc8b[:], rc8[:])
                    for hp2 in range(4 * g, 4 * g + 4):
                        j = hp2 % 4
                        rb = psO.tile([P, TQ], F32,
                                      tag=f"avps{hp2 % 2}", name="rb")
                        nc.tensor.matmul(
                            rb[:], sel_sb[:, j, :], rc8b[:],
                            start=True, stop=True)
                        nc.vector.tensor_mul(attnT[:, hp2, :],
                                             attnT[:, hp2, :], rb[:])

        if debug:
            nc.sync.dma_start(aT_d[:], attnT[:])

        # ---------------- Phase C: output projection ----------------
        with tc.tile_pool(name="wop", bufs=2) as pwo, \
                tc.tile_pool(name="outp", bufs=1) as pout, \
                tc.tile_pool(name="psC", bufs=3, space="PSUM") as psC:
            out_sb = pout.tile([P, NQT, DIM], F32, tag="out")
            wo_sbs = []
            for oh in range(2):
                w_sb = pwo.tile([P, NDC, 512], DT, tag="wo")
                nc.sync.dma_start(
                    w_sb[:],
                    woT[:, oh * 512:(oh + 1) * 512]
                    .rearrange("(adc p) o -> p adc o", p=P),
                )
                wo_sbs.append(w_sb)
            out_r = out.rearrange("(tt p) o -> p tt o", p=P)
            for tt in range(NQT):
                pss = [psC.tile([P, 512], F32, tag="cps", name="psc")
                       for _ in range(2)]
                for adc in range(NDC):
                    for oh in range(2):
                        nc.tensor.matmul(
                            pss[oh][:],
                            attnT[:, adc, tt * P:(tt + 1) * P],
                            wo_sbs[oh][:, adc, :],
                            start=(adc == 0), stop=(adc == NDC - 1),
                        )
                for oh in range(2):
                    nc.vector.tensor_copy(
                        out_sb[:, tt, oh * 512:(oh + 1) * 512], pss[oh][:])
                nc.sync.dma_start(out_r[:, tt, :], out_sb[:, tt, :])

    return nc


def _np_dt():
    return np.float32 if PREC == "fp32r" else ml_dtypes.bfloat16


def _build_ext(chunk0: bool):
    """qext [4,H,TQ]: (hi_q, lo_q, 1, 1);  kext [4,H,TKV]: (1, 1, k_hi, k_lo).
    hi/lo are a two-term bf16 split of -slope_h*(q_local+512); k_hi/k_lo of
    slope_h*k_local. Padded halo keys (chunk 0) get k_hi = -3e4 -> exp = 0."""
    np_dt = _np_dt()
    qe = np.zeros((4, H, TQ), np.float32)
    ke = np.zeros((4, H, TKV), np.float32)
    qpos = np.arange(TQ, dtype=np.float64) + 512.0
    kpos = np.arange(TKV, dtype=np.float64)
    qe[2:] = 1.0
    ke[:2] = 1.0
    qhi = np.zeros((H, TQ), np_dt)
    qlo = np.zeros((H, TQ), np_dt)
    khi = np.zeros((H, TKV), np_dt)
    klo = np.zeros((H, TKV), np_dt)
    for h in range(H):
        tq = (-SLOPES[h] * qpos).astype(np.float32)
        hi = tq.astype(np_dt)
        qhi[h] = hi
        qlo[h] = (tq - hi.astype(np.float32)).astype(np_dt)
        tk = (SLOPES[h] * kpos).astype(np.float32)
        if chunk0:
            tk[:512] = -3.0e4
        hi = tk.astype(np_dt)
        khi[h] = hi
        lo = (tk - hi.astype(np.float32))
        if chunk0:
            lo[:512] = 0.0
        klo[h] = lo.astype(np_dt)
    qe = qe.astype(np_dt)
    ke = ke.astype(np_dt)
    qe[0], qe[1] = qhi, qlo
    ke[2], ke[3] = khi, klo
    return np.ascontiguousarray(qe), np.ascontiguousarray(ke)


_NC = None
LAST = None  # BassKernelResults of the most recent run (exec_time_ns when traced)


def _get_nc():
    global _NC
    if _NC is None:
        _NC = _build_program()
    return _NC


def kernel(x, wq, wk, wv, wo, q_gamma, k_gamma):
    x = np.ascontiguousarray(np.asarray(x, np.float32))
    wq = np.asarray(wq, np.float32)
    wk = np.asarray(wk, np.float32)
    wv = np.asarray(wv, np.float32)
    wo = np.asarray(wo, np.float32)
    q_gamma = np.asarray(q_gamma, np.float32)
    k_gamma = np.asarray(k_gamma, np.float32)

    np_dt = _np_dt()
    wT_host = np.ascontiguousarray(
        np.concatenate([wq.T, wk.T, wv.T], axis=1).astype(np_dt))
    woT_host = np.ascontiguousarray(wo.T.astype(np_dt))
    gam_host = np.ascontiguousarray(np.concatenate(
        [q_gamma.reshape(NDC, P).T, k_gamma.reshape(NDC, P).T], axis=1))
    qe0, ke0 = _build_ext(True)
    qei, kei = _build_ext(False)

    in_maps = []
    for c in range(NCORES):
        b, j = divmod(c, 4)
        lo = j * TQ - WINDOW
        xs = x[b, max(0, lo): j * TQ + TQ, :]
        if lo < 0:
            xs = np.concatenate(
                [np.zeros((-lo, DIM), np.float32), xs], axis=0)
        in_maps.append({
            "nonce": np.zeros((1, _src_nonce()), np.float32),
            "xT": np.ascontiguousarray(xs.T.astype(np_dt)),
            "wT": wT_host,
            "woT": woT_host,
            "gam": gam_host,
            "qext": qe0 if j == 0 else qei,
            "kext": ke0 if j == 0 else kei,
        })

    global LAST
    trace = bool(int(os.environ.get("KERNEL_TRACE", "0") or 0))
    try:
        LAST = run_bass_kernel_spmd(
            _get_nc(), in_maps, list(range(NCORES)), trace=trace)
    except Exception:
        # a previously-wedged device surfaces as NRT_EXEC_UNIT_UNRECOVERABLE
        # on the first touch; reset the accelerator once and retry
        try:
            lib = ctypes.CDLL("/opt/axon/libaxon_pjrt.so")
            lib.axon_reset.restype = ctypes.c_int64
            import jax
            jax.devices()
            lib.axon_reset()
        except Exception:
            pass
        LAST = run_bass_kernel_spmd(
            _get_nc(), in_maps, list(range(NCORES)), trace=trace)

    full = np.empty((B, T, DIM), np.float32)
    for c in range(NCORES):
        b, j = divmod(c, 4)
        full[b, j * TQ:(j + 1) * TQ, :] = LAST.results[c]["out"]
    return full


# revision 23
# speedup vs baseline: 1.2179x; 1.2179x over previous
"""CodecAttention (sliding-window attention w/ QK-RMSNorm + ALiBi) on 8 trn2 cores.

Sharding: data-parallel over (batch, sequence-chunk): 2 batches x 4 chunks of 512
queries -> 8 cores. Each core recomputes K/V for its 512-token halo (zero-padded
for the first chunk), so there is no cross-core communication; the host only
slices/transposes inputs and concatenates the 8 disjoint output slices.

On-core pipeline (bf16 operands, fp32 PSUM accumulation):
  A) QKV projections from x^T (dim-on-partitions); QK RMSNorm over the flat
     1024-dim axis via ACT-square + accumulating ones-matmul partition
     reduction, rsqrt as exp(-0.5*ln(x)), 1/sqrt(dh) folded into the q scale.
  B) Head pairs: S^T tiles = k^T.T @ q^T as a 68-row contraction per head
     (64 head dims + 4 ALiBi rows: hi/lo bf16-split position terms; padded
     halo keys of chunk 0 killed via a -3e4 key bias), ACT exp
     with a fixed offset straight off PSUM (scores bounded), gpsimd
     affine_select/memset zero the causal/window boundary triangles and
     dead rectangles on the post-exp pt tiles, AV+rowsum with V-as-stationary
     (ones column yields softmax denominators), denominators DMA'd from PSUM
     into an [8, TQ] tile, one grouped reciprocal, K=8 selection-matrix
     matmul broadcasts the reciprocals per pair, per-pair normalize multiply.
  C) out = attnT.T @ wo^T per token tile, streamed 512KB output DMAs.
"""

import contextlib
import ctypes
import os
import sys
import types

import ml_dtypes
import numpy as np

import concourse.bass as bass
import concourse.mybir as mybir
import concourse.tile as tile


def _install_axon_ntff_shim():
    """bass_utils' trace path wants antenv.axon_hooks, which this image lacks.
    Provide it, backed by direct ctypes calls into libaxon_pjrt.so (same ABI
    the agent boot would use). Degrades to hook=None if the .so is absent."""
    try:
        import antenv.axon_hooks  # noqa: F401
        return
    except ImportError:
        pass

    _hook_holder = [None]
    so_path = "/opt/axon/libaxon_pjrt.so"
    if os.path.exists(so_path):
        try:
            lib = ctypes.CDLL(so_path)
            if hasattr(lib, "axon_start_nrt_profile"):
                lib.axon_start_nrt_profile.argtypes = [
                    ctypes.POINTER(ctypes.c_int64), ctypes.c_size_t]
                lib.axon_start_nrt_profile.restype = ctypes.c_int64
                lib.axon_stop_nrt_profile.argtypes = [ctypes.c_char_p]
                lib.axon_stop_nrt_profile.restype = ctypes.c_int64

                @contextlib.contextmanager
                def _hook(output_dir, device_ids):
                    import jax
                    jax.devices()
                    if device_ids:
                        ids = (ctypes.c_int64 * len(device_ids))(*device_ids)
                        rc = lib.axon_start_nrt_profile(ids, len(device_ids))
                    else:
                        rc = lib.axon_start_nrt_profile(None, 0)
                    if rc != 0:
                        raise RuntimeError(f"axon_start_nrt_profile rc={rc}")
                    try:
                        yield
                    finally:
                        n = lib.axon_stop_nrt_profile(str(output_dir).encode())
                        if n < 0:
                            raise RuntimeError(f"axon_stop_nrt_profile rc={n}")

                _hook_holder[0] = _hook
        except OSError:
            pass

    mod = types.ModuleType("antenv.axon_hooks")
    mod.get_axon_ntff_profile_hook = lambda: _hook_holder[0]
    mod.set_axon_ntff_profile_hook = lambda h: _hook_holder.__setitem__(0, h)
    sys.modules["antenv.axon_hooks"] = mod


_install_axon_ntff_shim()

from concourse.bass_utils import run_bass_kernel_spmd  # noqa: E402
from bass_rust import ScopedClock  # noqa: E402

B, T, DIM = 2, 2048, 1024
H, DH, WINDOW = 16, 64, 512
P = 128
TQ = 512            # queries per core
TKV = 1024          # kv tokens per core (incl. 512 halo)
NCORES = 8
NQT = TQ // P       # 4
NKT = TKV // P      # 8
NDC = DIM // P      # 8
EXP_C = 10.0        # exp offset; true max masked score is ~6.0 for this data
F32 = mybir.dt.float32
F32R = mybir.dt.float32r
BF16 = mybir.dt.bfloat16
PREC = os.environ.get("KERNEL_PREC", "bf16")
DT = F32R if PREC == "fp32r" else BF16
AF = mybir.ActivationFunctionType
ALU = mybir.AluOpType

SLOPES = [2.0 ** (-0.5 * (h + 1)) for h in range(H)]

# Score-tile packing: per head, scores are computed as S^T [key, query] in two
# 3-bank PSUM halves of [128, 1536]. Key-tile kt covers queries
# [KT_QLO[kt], KT_QLO[kt]+KT_W[kt]) at column offset KT_OFF[kt] of its half.
KT_W = [256, 256, 512, 512, 512, 512, 256, 256]
KT_OFF = [0, 256, 512, 1024, 0, 512, 1024, 1280]
KT_QLO = [0, 0, 0, 0, 0, 0, 256, 256]
HW_HALF = 1536
# post-exp boundary cleanup per half: (col0, kind) with kind 'win' (keep
# key>=query within the block), 'causal' (keep query>=key), 'dead' (zero all).
# Block (kt, qb) is valid iff qb <= kt <= qb+4; kt==qb is the window edge,
# kt==qb+4 the causal edge; anything else computed by the packing is dead.
PT_FIX = {
    0: [(0, "win"), (128, "dead"), (256 + 128, "win"),
        (512 + 256, "win"), (512 + 384, "dead"), (1024 + 384, "win")],
    1: [(0, "causal"), (512, "dead"), (512 + 128, "causal"),
        (1024, "causal"), (1280, "dead"), (1280 + 128, "causal")],
}
# 256-query output blocks: which key tiles contribute to each
QB_KTS = {0: [0, 1, 2, 3, 4, 5], 1: [2, 3, 4, 5, 6, 7]}
# merged AV matmul plan: (kt, out_col_lo, width, start, stop)
AV_PLAN = [
    (0, 0, 256, True, False),
    (1, 0, 256, False, False),
    (2, 0, 256, False, False), (2, 256, 256, True, False),
    (3, 0, 512, False, False),
    (4, 0, 512, False, False),
    (5, 0, 512, False, False),
    (6, 256, 256, False, False),
    (7, 256, 256, False, True),
]


class _SplitDrainTileContext(tile.TileContext):
    """The walrus build in this env rejects >1-2 sync-wait commands on one
    instruction; spread excess waits across same-engine NOPs placed directly
    before the over-limit instruction (per-engine program order preserved)."""

    def _split_excess_waits(self):
        nc = self.nc
        cur_list = nc.cur_bb.bb.instructions
        for blk in nc.m.functions[0].blocks:
            snapshot = list(blk.instructions)
            for inst in snapshot:
                si = inst.sync_info
                max_w = 1
                if si is None or len(si.on_wait) <= max_w:
                    continue
                waits = list(si.on_wait)
                si.on_wait = waits[:max_w]
                eng_obj = nc.engines[inst.engine]
                for w in waits[max_w:]:
                    nop_bi = eng_obj.nop(nofuse=True, hint="wait_split")
                    nop_inst = nop_bi.ins
                    nop_inst.sync_info = mybir.SyncInfo(on_wait=[w], on_update=[])
                    cur_list.remove(nop_inst)
                    blk.instructions.insert(
                        blk.instructions.index(inst), nop_inst)

    def _drain_and_barrier(self, tick_clock, wait_clock):
        self._split_excess_waits()
        drain_inst = self.nc.sync.drain()
        wait_clock.add_sem_waits(
            drain_inst.ins, ScopedClock({None: tick_clock.global_clock})
        )
        si = drain_inst.ins.sync_info
        if si is not None and len(si.on_wait) > 1:
            waits = list(si.on_wait)
            si.on_wait = waits[:1]
            for w in waits[1:]:
                nop = self.nc.sync.nop(nofuse=True, hint="drain_wait_split")
                nop.ins.sync_info = mybir.SyncInfo(on_wait=[w], on_update=[])
        self.nc.all_engine_barrier()
        assert self.sems is not None
        popped = self.nc._tile_sem_poison_stack.pop()
        assert popped is self._sem_poison
        self.nc.clear_and_free_semaphores(list(self.sems.allocated().values()))
        self.nc.all_engine_barrier()


def _src_nonce():
    import zlib
    with open(__file__, "rb") as f:
        return (zlib.crc32(f.read() + PREC.encode()) % 2048) + 8


def _pt_fixups(nc, pt, half):
    """Zero the invalid regions of a post-exp pt tile on the gpsimd engine."""
    for col0, kind in PT_FIX[half]:
        sl = pt[:, col0:col0 + P]
        if kind == "dead":
            nc.gpsimd.memset(sl, 0.0)
        elif kind == "win":
            # keep iff key_local >= query_local  (iota = p - qi >= 0)
            nc.gpsimd.affine_select(
                out=sl, in_=sl, compare_op=ALU.is_ge, fill=0.0,
                base=0, pattern=[[-1, P]], channel_multiplier=1)
        else:
            # causal: keep iff query_local >= key_local (iota = qi - p >= 0)
            nc.gpsimd.affine_select(
                out=sl, in_=sl, compare_op=ALU.is_ge, fill=0.0,
                base=0, pattern=[[1, P]], channel_multiplier=-1)


def _build_program(debug=False):
    nc = bass.Bass()
    # dummy input whose shape changes with this file: busts HLO-keyed NEFF
    # caches (the BIR itself is not part of the HLO fingerprint)
    nonce = nc.declare_dram_parameter("nonce", [1, _src_nonce()], F32,
                                      isOutput=False)
    xT = nc.declare_dram_parameter("xT", [DIM, TKV], DT, isOutput=False)
    wT = nc.declare_dram_parameter("wT", [DIM, 3 * DIM], DT, isOutput=False)
    woT = nc.declare_dram_parameter("woT", [DIM, DIM], DT, isOutput=False)
    qext = nc.declare_dram_parameter("qext", [4, H, TQ], DT, isOutput=False)
    kext = nc.declare_dram_parameter("kext", [4, H, TKV], DT, isOutput=False)
    gam = nc.declare_dram_parameter("gam", [DH, 2 * H], F32, isOutput=False)
    out = nc.declare_dram_parameter("out", [TQ, DIM], F32, isOutput=True)
    if debug:
        qT_d = nc.declare_dram_parameter("qT_d", [P, NDC, TQ], DT, isOutput=True)
        kT_d = nc.declare_dram_parameter("kT_d", [P, NDC, TKV], DT, isOutput=True)
        V_d = nc.declare_dram_parameter("V_d", [P, NKT, H, DH + 1], DT, isOutput=True)
        aT_d = nc.declare_dram_parameter("aT_d", [P, NDC, TQ], DT, isOutput=True)
        pt_d = nc.declare_dram_parameter("pt_d", [2, P, 2, HW_HALF], DT, isOutput=True)

    with _SplitDrainTileContext(nc) as tc, \
            tc.tile_pool(name="persist", bufs=1) as pp, \
            tc.tile_pool(name="small", bufs=1) as psm:

        # per-head score operands: rows 0..63 = head dims, rows 64..67 = the
        # ALiBi rank-4 rows (folded into the same 68-partition contraction)
        qT = pp.tile([P, H, TQ], DT, tag="qT")         # [p, h, tok]
        kT = pp.tile([P, H, TKV], DT, tag="kT")
        V = pp.tile([P, NKT, H, DH + 1], DT, tag="V")  # [p=tok, kt, h, dh+ones]
        attnT = pp.tile([P, NDC, TQ], DT, tag="attnT")
        gam_sb = pp.tile([DH, 2 * H], F32, tag="gam")
        nc.sync.dma_start(gam_sb[:], gam[:])
        ones_sb = pp.tile([P, 1], F32R, tag="ones")
        ones_row = pp.tile([1, P], F32, tag="onesrow")
        negc_sb = pp.tile([P, 1], F32, tag="negc")
        eps_sb = pp.tile([1, 1], F32, tag="eps")
        ln8_sb = pp.tile([1, 1], F32, tag="ln8")
        # per-pair selection matrices for the reciprocal broadcast matmul:
        # sel[j][g, p] = 1 iff g == 2j + p//64   (K=8 stationary, bf16)
        sel_sb = pp.tile([8, 4, P], BF16, tag="sel")
        nc.vector.memset(ones_sb[:].bitcast(F32), 1.0)
        nc.vector.memset(ones_row[:], 1.0)
        nc.vector.memset(negc_sb[:], -EXP_C)
        nc.vector.memset(eps_sb[:], 1.0e-6)
        nc.vector.memset(ln8_sb[:], float(-0.5 * np.log(64.0)))
        nc.gpsimd.memset(sel_sb[:], 0.0)
        for j in range(4):
            nc.gpsimd.affine_select(
                out=sel_sb[:, j, :], in_=sel_sb[:, j, :],
                compare_op=ALU.not_equal, fill=1.0,
                base=-2 * j, pattern=[[-1, 2], [0, 64]], channel_multiplier=1)
        nonce_sb = pp.tile([1, _src_nonce()], F32, tag="nonce")
        nc.sync.dma_start(nonce_sb[:], nonce[:])
        nc.sync.dma_start(qT[64:68], qext[:])
        nc.sync.dma_start(kT[64:68], kext[:])
        ones_col = V[:, :, :, DH]
        nc.vector.memset(
            ones_col if DT == BF16 else ones_col.bitcast(F32), 1.0)

        # ---------------- Phase A: projections + RMSNorm ----------------
        with tc.tile_pool(name="xp", bufs=1) as px, \
                tc.tile_pool(name="wp", bufs=2) as pw, \
                tc.tile_pool(name="sqp", bufs=2) as psq, \
                tc.tile_pool(name="accp", bufs=1) as pacc, \
                tc.tile_pool(name="psA", bufs=4, space="PSUM") as psA, \
                tc.tile_pool(name="psS1", bufs=2, space="PSUM") as psS1, \
                tc.tile_pool(name="psBC", bufs=2, space="PSUM") as psBC:

            x_sb = px.tile([P, NDC, TKV], DT, tag="x")
            wq0_sb = pw.tile([P, NDC, 512], DT, tag="wslice", name="wq0")
            wq0_src = wT[:, 0:512].rearrange("(dc p) o -> p dc o", p=P)
            # priority order: x tokens 512.. (Q + K-group-2 operands) first
            for dc in range(NDC):
                nc.sync.dma_start(x_sb[:, dc, 512:1024],
                                  xT[dc * P:(dc + 1) * P, 512:1024])
            for dc in range(NDC):
                nc.sync.dma_start(wq0_sb[:, dc, :], wq0_src[:, dc, :])
                nc.sync.dma_start(x_sb[:, dc, 0:512],
                                  xT[dc * P:(dc + 1) * P, 0:512])

            # Q (tokens 512..1023 of the kv range) and K (all tokens);
            # K group (512,512) first so it only needs the priority x half
            sqacc = {}
            for proj in range(2):
                dst = qT if proj == 0 else kT
                groups = [(TKV - TQ, 0)] if proj == 0 else [(512, 512), (0, 0)]
                for wh in range(2):
                    if proj == 0 and wh == 0:
                        w_sb = wq0_sb
                    else:
                        w_sb = pw.tile([P, NDC, 512], DT, tag="wslice")
                        nc.sync.dma_start(
                            w_sb[:],
                            wT[:, proj * DIM + wh * 512: proj * DIM + (wh + 1) * 512]
                            .rearrange("(dc p) o -> p dc o", p=P),
                        )
                    for ol in range(4):
                        odt = wh * 4 + ol
                        for (soff, doff) in groups:
                            ps = psA.tile([P, 512], F32, tag="projps")
                            for dc in range(NDC):
                                nc.tensor.matmul(
                                    ps[:],
                                    w_sb[:, dc, ol * P:(ol + 1) * P],
                                    x_sb[:, dc, soff:soff + 512],
                                    start=(dc == 0), stop=(dc == NDC - 1),
                                )
                            for hi in range(2):
                                nc.scalar.copy(
                                    dst[0:DH, 2 * odt + hi, doff:doff + 512],
                                    ps[DH * hi:DH * (hi + 1), :])
                            sq = psq.tile([P, 512], F32, tag="sq")
                            nc.scalar.activation(sq[:], ps[:], AF.Square)
                            key = (proj, doff)
                            if odt == 0:
                                acc = pacc.tile([P, 512], F32,
                                                tag=f"acc{proj}_{doff}",
                                                name="acc")
                                sqacc[key] = acc
                                nc.vector.tensor_copy(acc[:], sq[:])
                            else:
                                nc.vector.tensor_add(sqacc[key][:],
                                                     sqacc[key][:], sq[:])

            # V projection: [tok, head, dh]; vh innermost so consecutive
            # matmuls share the x-chunk stationary operand (ldw-opt dedups);
            # token tiles 4..7 first (their x half arrives first)
            wv_sb = []
            for vh in range(2):
                w_sb = pw.tile([P, NDC, 512], DT, tag="wslice")
                nc.sync.dma_start(
                    w_sb[:],
                    wT[:, 2 * DIM + vh * 512: 2 * DIM + (vh + 1) * 512]
                    .rearrange("(dc p) o -> p dc o", p=P),
                )
                wv_sb.append(w_sb)
            for tt in (4, 5, 6, 7, 0, 1, 2, 3):
                pss = [psA.tile([P, 512], F32, tag="projps", name="psv")
                       for _ in range(2)]
                for dc in range(NDC):
                    for vh in range(2):
                        nc.tensor.matmul(
                            pss[vh][:],
                            x_sb[:, dc, tt * P:(tt + 1) * P],
                            wv_sb[vh][:, dc, :],
                            start=(dc == 0), stop=(dc == NDC - 1),
                        )
                for vh in range(2):
                    nc.scalar.copy(
                        V[:, tt, vh * 8:(vh + 1) * 8, :DH],
                        pss[vh][:].rearrange("p (h c) -> p h c", c=DH),
                    )

            # rsqrt(mean+eps) = exp(-0.5*ln(ss/DIM + eps)); the 1/sqrt(dh)
            # score scale folds into the exp bias for q
            bcasts = {}
            for (proj, doff), acc in sqacc.items():
                ss = psS1.tile([1, 512], F32, tag="ssq", name="ssq")
                nc.tensor.matmul(ss[:], ones_sb[:].bitcast(F32), acc[:],
                                 start=True, stop=True)
                a = psm.tile([1, 512], F32, tag="a")
                nc.scalar.activation(a[:], ss[:], AF.Ln,
                                     bias=eps_sb[:], scale=1.0 / DIM)
                y = psm.tile([1, 512], F32, tag="y")
                nc.scalar.activation(y[:], a[:], AF.Exp,
                                     bias=(ln8_sb[:] if proj == 0 else 0.0),
                                     scale=-0.5)
                # broadcast over partitions via K=1 ones-matmul (plain fp32)
                bc = psBC.tile([P, 512], F32, tag="bc", name="bc")
                nc.tensor.matmul(bc[:], ones_row[:], y[:], start=True, stop=True)
                bcasts[(proj, doff)] = bc

            # normalize in place (x gamma); head-major order so head pair 0
            # becomes ready first
            for odt in range(NDC):
                for proj in range(2):
                    dst = qT if proj == 0 else kT
                    for hi in range(2):
                        h = 2 * odt + hi
                        gap = gam_sb[:, proj * H + h: proj * H + h + 1]
                        for doff in ([0] if proj == 0 else [0, 512]):
                            sl = dst[0:DH, h, doff:doff + 512]
                            nc.vector.scalar_tensor_tensor(
                                sl, sl, gap,
                                bcasts[(proj, doff)][0:DH, :],
                                op0=ALU.mult, op1=ALU.mult,
                            )

        if debug:
            nc.sync.dma_start(qT_d[:], qT[:])
            nc.sync.dma_start(kT_d[:], kT[:])
            nc.sync.dma_start(V_d[:], V[:])

        # ---------------- Phase B: attention (head pairs) ----------------
        with tc.tile_pool(name="maskp", bufs=1) as pm, \
                tc.tile_pool(name="ptp", bufs=2) as ppt, \
                tc.tile_pool(name="rnp", bufs=1) as prn, \
                tc.tile_pool(name="psS", bufs=1, space="PSUM") as psS, \
                tc.tile_pool(name="psO", bufs=1, space="PSUM") as psO:

            s16 = [pm.tile([8, TQ], F32, tag="s16a", name="s16a"),
                   pm.tile([8, TQ], F32, tag="s16b", name="s16b")]

            for hp in range(NDC):
                ps_o = [psO.tile([DH + 1, TQ], F32, tag=f"avps{hi}", name=f"avps{hi}")
                        for hi in range(2)]
                pts = {0: [], 1: []}
                for half in range(2):
                    ps_pair = [psS.tile([P, HW_HALF], F32, tag=f"sps{hi}", name=f"sps{hi}")
                               for hi in range(2)]
                    for ktl in range(4):
                        kt = half * 4 + ktl
                        off, wdt, qlo = KT_OFF[kt], KT_W[kt], KT_QLO[kt]
                        for hi in range(2):
                            h = 2 * hp + hi
                            # 68-row contraction: head dims + ALiBi rank-4
                            nc.tensor.matmul(
                                ps_pair[hi][:, off:off + wdt],
                                kT[0:DH + 4, h, kt * P:(kt + 1) * P],
                                qT[0:DH + 4, h, qlo:qlo + wdt],
                                start=True, stop=True,
                            )
                    for hi in range(2):
                        pt = ppt.tile([P, HW_HALF], DT, tag=f"pt{hi}")
                        nc.scalar.activation(pt[:], ps_pair[hi][:], AF.Exp,
                                             bias=negc_sb[:])
                        _pt_fixups(nc, pt, half)
                        if debug and hp == 0:
                            nc.sync.dma_start(pt_d[hi, :, half, :], pt[:])
                        pts[hi].append(pt)

                for hi in range(2):
                    h = 2 * hp + hi
                    po = DH * hi
                    for qb in range(2):
                        kts = QB_KTS[qb]
                        for i, kt in enumerate(kts):
                            half, off, qlo = kt // 4, KT_OFF[kt], KT_QLO[kt]
                            c0 = off + qb * 256 - qlo
                            nc.tensor.matmul(
                                ps_o[hi][:, qb * 256:(qb + 1) * 256],
                                V[:, kt, h, :],
                                pts[hi][half][:, c0:c0 + 256],
                                start=(i == 0), stop=(i == len(kts) - 1),
                            )
                    # stash the softmax denominator row; head h -> partition
                    # h of s16 via a tiny SBUF->SBUF DMA (engines cannot write
                    # partition offsets other than 0/32/64)
                    r = prn.tile([1, TQ], F32, tag=f"r{hi}", name="r")
                    nc.vector.tensor_copy(r[:], ps_o[hi][DH:DH + 1, :])
                    nc.sync.dma_start(s16[h // 8][h % 8:h % 8 + 1, :], r[:])
                    nc.vector.tensor_copy(attnT[po:po + DH, hp, :], ps_o[hi][:DH, :])

                if hp % 4 == 3:
                    # normalize the finished half (8 heads): batched reciprocal
                    # on 8 partitions, K=8 selection-matrix matmul broadcasts
                    # each pair's rows across its 128 partitions
                    g = hp // 4
                    rc8 = pm.tile([8, TQ], F32, tag=f"rc{g}", name="rc8")
                    nc.vector.reciprocal(rc8[:], s16[g][:])
                    rc8b = pm.tile([8, TQ], BF16, tag=f"rcb{g}", name="rc8b")
                    nc.vector.tensor_copy(rc8b[:], rc8[:])
                    for hp2 in range(4 * g, 4 * g + 4):
                        j = hp2 % 4
                        rb = psO.tile([P, TQ], F32,
                                      tag=f"avps{hp2 % 2}", name="rb")
                        nc.tensor.matmul(
                            rb[:], sel_sb[:, j, :], rc8b[:],
                            start=True, stop=True)
                        nc.vector.tensor_mul(attnT[:, hp2, :],
                                             attnT[:, hp2, :], rb[:])

        if debug:
            nc.sync.dma_start(aT_d[:], attnT[:])

        # ---------------- Phase C: output projection ----------------
        with tc.tile_pool(name="wop", bufs=2) as pwo, \
                tc.tile_pool(name="outp", bufs=1) as pout, \
                tc.tile_pool(name="psC", bufs=3, space="PSUM") as psC:
            out_sb = pout.tile([P, NQT, DIM], F32, tag="out")
            wo_sbs = []
            for oh in range(2):
                w_sb = pwo.tile([P, NDC, 512], DT, tag="wo")
                nc.sync.dma_start(
                    w_sb[:],
                    woT[:, oh * 512:(oh + 1) * 512]
                    .rearrange("(adc p) o -> p adc o", p=P),
                )
                wo_sbs.append(w_sb)
            out_r = out.rearrange("(tt p) o -> p tt o", p=P)
            for tt in range(NQT):
                pss = [psC.tile([P, 512], F32, tag="cps", name="psc")
                       for _ in range(2)]
                for adc in range(NDC):
                    for oh in range(2):
                        nc.tensor.matmul(
                            pss[oh][:],
                            attnT[:, adc, tt * P:(tt + 1) * P],
                            wo_sbs[oh][:, adc, :],
                            start=(adc == 0), stop=(adc == NDC - 1),
                        )
                for oh in range(2):
                    nc.vector.tensor_copy(
                        out_sb[:, tt, oh * 512:(oh + 1) * 512], pss[oh][:])
                nc.sync.dma_start(out_r[:, tt, :], out_sb[:, tt, :])

    return nc


def _np_dt():
    return np.float32 if PREC == "fp32r" else ml_dtypes.bfloat16


def _build_ext(chunk0: bool):
    """qext [4,H,TQ]: (hi_q, lo_q, 1, 1);  kext [4,H,TKV]: (1, 1, k_hi, k_lo).
    hi/lo are a two-term bf16 split of -slope_h*(q_local+512); k_hi/k_lo of
    slope_h*k_local. Padded halo keys (chunk 0) get k_hi = -3e4 -> exp = 0."""
    np_dt = _np_dt()
    qe = np.zeros((4, H, TQ), np.float32)
    ke = np.zeros((4, H, TKV), np.float32)
    qpos = np.arange(TQ, dtype=np.float64) + 512.0
    kpos = np.arange(TKV, dtype=np.float64)
    qe[2:] = 1.0
    ke[:2] = 1.0
    qhi = np.zeros((H, TQ), np_dt)
    qlo = np.zeros((H, TQ), np_dt)
    khi = np.zeros((H, TKV), np_dt)
    klo = np.zeros((H, TKV), np_dt)
    for h in range(H):
        tq = (-SLOPES[h] * qpos).astype(np.float32)
        hi = tq.astype(np_dt)
        qhi[h] = hi
        qlo[h] = (tq - hi.astype(np.float32)).astype(np_dt)
        tk = (SLOPES[h] * kpos).astype(np.float32)
        if chunk0:
            tk[:512] = -3.0e4
        hi = tk.astype(np_dt)
        khi[h] = hi
        lo = (tk - hi.astype(np.float32))
        if chunk0:
            lo[:512] = 0.0
        klo[h] = lo.astype(np_dt)
    qe = qe.astype(np_dt)
    ke = ke.astype(np_dt)
    qe[0], qe[1] = qhi, qlo
    ke[2], ke[3] = khi, klo
    return np.ascontiguousarray(qe), np.ascontiguousarray(ke)


_NC = None
LAST = None  # BassKernelResults of the most recent run (exec_time_ns when traced)


def _get_nc():
    global _NC
    if _NC is None:
        _NC = _build_program()
    return _NC


def kernel(x, wq, wk, wv, wo, q_gamma, k_gamma):
    x = np.ascontiguousarray(np.asarray(x, np.float32))
    wq = np.asarray(wq, np.float32)
    wk = np.asarray(wk, np.float32)
    wv = np.asarray(wv, np.float32)
    wo = np.asarray(wo, np.float32)
    q_gamma = np.asarray(q_gamma, np.float32)
    k_gamma = np.asarray(k_gamma, np.float32)

    np_dt = _np_dt()
    wT_host = np.ascontiguousarray(
        np.concatenate([wq.T, wk.T, wv.T], axis=1).astype(np_dt))
    woT_host = np.ascontiguousarray(wo.T.astype(np_dt))
    gam_host = np.ascontiguousarray(np.concatenate(
        [q_gamma.reshape(H, DH).T, k_gamma.reshape(H, DH).T], axis=1))
    qe0, ke0 = _build_ext(True)
    qei, kei = _build_ext(False)

    in_maps = []
    for c in range(NCORES):
        b, j = divmod(c, 4)
        lo = j * TQ - WINDOW
        xs = x[b, max(0, lo): j * TQ + TQ, :]
        if lo < 0:
            xs = np.concatenate(
                [np.zeros((-lo, DIM), np.float32), xs], axis=0)
        in_maps.append({
            "nonce": np.zeros((1, _src_nonce()), np.float32),
            "xT": np.ascontiguousarray(xs.T.astype(np_dt)),
            "wT": wT_host,
            "woT": woT_host,
            "gam": gam_host,
            "qext": qe0 if j == 0 else qei,
            "kext": ke0 if j == 0 else kei,
        })

    global LAST
    trace = bool(int(os.environ.get("KERNEL_TRACE", "0") or 0))
    try:
        LAST = run_bass_kernel_spmd(
            _get_nc(), in_maps, list(range(NCORES)), trace=trace)
    except Exception:
        # a previously-wedged device surfaces as NRT_EXEC_UNIT_UNRECOVERABLE
        # on the first touch; reset the accelerator once and retry
        try:
            lib = ctypes.CDLL("/opt/axon/libaxon_pjrt.so")
            lib.axon_reset.restype = ctypes.c_int64
            import jax
            jax.devices()
            lib.axon_reset()
        except Exception:
            pass
        LAST = run_bass_kernel_spmd(
            _get_nc(), in_maps, list(range(NCORES)), trace=trace)

    full = np.empty((B, T, DIM), np.float32)
    for c in range(NCORES):
        b, j = divmod(c, 4)
        full[b, j * TQ:(j + 1) * TQ, :] = LAST.results[c]["out"]
    return full


# revision 33
# speedup vs baseline: 1.2491x; 1.0256x over previous
"""CodecAttention (sliding-window attention w/ QK-RMSNorm + ALiBi) on 8 trn2 cores.

Sharding: data-parallel over (batch, sequence-chunk): 2 batches x 4 chunks of 512
queries -> 8 cores. Each core recomputes K/V for its 512-token halo (zero-padded
for the first chunk), so there is no cross-core communication; the host only
slices/transposes inputs and concatenates the 8 disjoint output slices.

On-core pipeline (bf16 operands, fp32 PSUM accumulation):
  A) QKV projections from x^T (dim-on-partitions); QK RMSNorm over the flat
     1024-dim axis via ACT-square + accumulating ones-matmul partition
     reduction, rsqrt as exp(-0.5*ln(x)), 1/sqrt(dh) folded into the q scale.
  B) Head pairs: S^T tiles = k^T.T @ q^T as a 68-row contraction per head
     (64 head dims + 4 ALiBi rows: hi/lo bf16-split position terms; padded
     halo keys of chunk 0 killed via a -3e4 key bias), ACT exp
     with a fixed offset straight off PSUM (scores bounded), gpsimd
     affine_select/memset zero the causal/window boundary triangles and
     dead rectangles on the post-exp pt tiles, AV+rowsum with V-as-stationary
     (ones column yields softmax denominators), denominators DMA'd from PSUM
     into an [8, TQ] tile, one grouped reciprocal, K=8 selection-matrix
     matmul broadcasts the reciprocals per pair, per-pair normalize multiply.
  C) out = attnT.T @ wo^T per token tile, streamed 512KB output DMAs.
"""

import contextlib
import ctypes
import os
import sys
import types

import ml_dtypes
import numpy as np

import concourse.bass as bass
import concourse.mybir as mybir
import concourse.tile as tile


def _install_axon_ntff_shim():
    """bass_utils' trace path wants antenv.axon_hooks, which this image lacks.
    Provide it, backed by direct ctypes calls into libaxon_pjrt.so (same ABI
    the agent boot would use). Degrades to hook=None if the .so is absent."""
    try:
        import antenv.axon_hooks  # noqa: F401
        return
    except ImportError:
        pass

    _hook_holder = [None]
    so_path = "/opt/axon/libaxon_pjrt.so"
    if os.path.exists(so_path):
        try:
            lib = ctypes.CDLL(so_path)
            if hasattr(lib, "axon_start_nrt_profile"):
                lib.axon_start_nrt_profile.argtypes = [
                    ctypes.POINTER(ctypes.c_int64), ctypes.c_size_t]
                lib.axon_start_nrt_profile.restype = ctypes.c_int64
                lib.axon_stop_nrt_profile.argtypes = [ctypes.c_char_p]
                lib.axon_stop_nrt_profile.restype = ctypes.c_int64

                @contextlib.contextmanager
                def _hook(output_dir, device_ids):
                    import jax
                    jax.devices()
                    if device_ids:
                        ids = (ctypes.c_int64 * len(device_ids))(*device_ids)
                        rc = lib.axon_start_nrt_profile(ids, len(device_ids))
                    else:
                        rc = lib.axon_start_nrt_profile(None, 0)
                    if rc != 0:
                        raise RuntimeError(f"axon_start_nrt_profile rc={rc}")
                    try:
                        yield
                    finally:
                        n = lib.axon_stop_nrt_profile(str(output_dir).encode())
                        if n < 0:
                            raise RuntimeError(f"axon_stop_nrt_profile rc={n}")

                _hook_holder[0] = _hook
        except OSError:
            pass

    mod = types.ModuleType("antenv.axon_hooks")
    mod.get_axon_ntff_profile_hook = lambda: _hook_holder[0]
    mod.set_axon_ntff_profile_hook = lambda h: _hook_holder.__setitem__(0, h)
    sys.modules["antenv.axon_hooks"] = mod


_install_axon_ntff_shim()

from concourse.bass_utils import run_bass_kernel_spmd  # noqa: E402
from bass_rust import ScopedClock  # noqa: E402

B, T, DIM = 2, 2048, 1024
H, DH, WINDOW = 16, 64, 512
P = 128
TQ = 512            # queries per core
TKV = 1024          # kv tokens per core (incl. 512 halo)
NCORES = 8
NQT = TQ // P       # 4
NKT = TKV // P      # 8
NDC = DIM // P      # 8
EXP_C = 10.0        # exp offset; true max masked score is ~6.0 for this data
F32 = mybir.dt.float32
F32R = mybir.dt.float32r
BF16 = mybir.dt.bfloat16
PREC = os.environ.get("KERNEL_PREC", "bf16")
DT = F32R if PREC == "fp32r" else BF16
AF = mybir.ActivationFunctionType
ALU = mybir.AluOpType

SLOPES = [2.0 ** (-0.5 * (h + 1)) for h in range(H)]

# Score-tile packing: per head, scores are computed as S^T [key, query] in two
# 3-bank PSUM halves of [128, 1536]. Key-tile kt covers queries
# [KT_QLO[kt], KT_QLO[kt]+KT_W[kt]) at column offset KT_OFF[kt] of its half.
KT_W = [256, 256, 512, 512, 512, 512, 256, 256]
KT_OFF = [0, 256, 512, 1024, 0, 512, 1024, 1280]
KT_QLO = [0, 0, 0, 0, 0, 0, 256, 256]
HW_HALF = 1536
# post-exp boundary cleanup per half: (col0, kind) with kind 'win' (keep
# key>=query within the block), 'causal' (keep query>=key), 'dead' (zero all).
# Block (kt, qb) is valid iff qb <= kt <= qb+4; kt==qb is the window edge,
# kt==qb+4 the causal edge; anything else computed by the packing is dead.
PT_FIX = {
    0: [(0, "win"), (128, "dead"), (256 + 128, "win"),
        (512 + 256, "win"), (512 + 384, "dead"), (1024 + 384, "win")],
    1: [(0, "causal"), (512, "dead"), (512 + 128, "causal"),
        (1024, "causal"), (1280, "dead"), (1280 + 128, "causal")],
}
# merged AV matmul plan: (kt, out_col_lo, width); the four middle key tiles
# serve all 512 queries in one 512-wide matmul, edge tiles only their 256-
# query block (per-element PSUM has_written handles the region accumulation;
# the first, full-width matmul initializes every element)
AV_PLAN = [
    (2, 0, 512), (3, 0, 512), (4, 0, 512), (5, 0, 512),
    (0, 0, 256), (1, 0, 256), (6, 256, 256), (7, 256, 256),
]


class _SplitDrainTileContext(tile.TileContext):
    """The walrus build in this env rejects >1-2 sync-wait commands on one
    instruction; spread excess waits across same-engine NOPs placed directly
    before the over-limit instruction (per-engine program order preserved)."""

    def _split_excess_waits(self):
        nc = self.nc
        cur_list = nc.cur_bb.bb.instructions
        for blk in nc.m.functions[0].blocks:
            snapshot = list(blk.instructions)
            for inst in snapshot:
                si = inst.sync_info
                max_w = 1
                if si is None or len(si.on_wait) <= max_w:
                    continue
                waits = list(si.on_wait)
                si.on_wait = waits[:max_w]
                eng_obj = nc.engines[inst.engine]
                for w in waits[max_w:]:
                    nop_bi = eng_obj.nop(nofuse=True, hint="wait_split")
                    nop_inst = nop_bi.ins
                    nop_inst.sync_info = mybir.SyncInfo(on_wait=[w], on_update=[])
                    cur_list.remove(nop_inst)
                    blk.instructions.insert(
                        blk.instructions.index(inst), nop_inst)

    def _drain_and_barrier(self, tick_clock, wait_clock):
        self._split_excess_waits()
        drain_inst = self.nc.sync.drain()
        wait_clock.add_sem_waits(
            drain_inst.ins, ScopedClock({None: tick_clock.global_clock})
        )
        si = drain_inst.ins.sync_info
        if si is not None and len(si.on_wait) > 1:
            waits = list(si.on_wait)
            si.on_wait = waits[:1]
            for w in waits[1:]:
                nop = self.nc.sync.nop(nofuse=True, hint="drain_wait_split")
                nop.ins.sync_info = mybir.SyncInfo(on_wait=[w], on_update=[])
        self.nc.all_engine_barrier()
        assert self.sems is not None
        popped = self.nc._tile_sem_poison_stack.pop()
        assert popped is self._sem_poison
        self.nc.clear_and_free_semaphores(list(self.sems.allocated().values()))
        self.nc.all_engine_barrier()


def _src_nonce():
    import zlib
    with open(__file__, "rb") as f:
        return (zlib.crc32(f.read() + PREC.encode()) % 2048) + 8


def _pt_fixups(nc, pt, half):
    """Zero the invalid regions of a post-exp pt tile on the gpsimd engine."""
    for col0, kind in PT_FIX[half]:
        sl = pt[:, col0:col0 + P]
        if kind == "dead":
            nc.gpsimd.memset(sl, 0.0)
        elif kind == "win":
            # keep iff key_local >= query_local  (iota = p - qi >= 0)
            nc.gpsimd.affine_select(
                out=sl, in_=sl, compare_op=ALU.is_ge, fill=0.0,
                base=0, pattern=[[-1, P]], channel_multiplier=1)
        else:
            # causal: keep iff query_local >= key_local (iota = qi - p >= 0)
            nc.gpsimd.affine_select(
                out=sl, in_=sl, compare_op=ALU.is_ge, fill=0.0,
                base=0, pattern=[[1, P]], channel_multiplier=-1)


def _build_program(debug=False):
    nc = bass.Bass()
    # dummy input whose shape changes with this file: busts HLO-keyed NEFF
    # caches (the BIR itself is not part of the HLO fingerprint)
    nonce = nc.declare_dram_parameter("nonce", [1, _src_nonce()], F32,
                                      isOutput=False)
    xT = nc.declare_dram_parameter("xT", [DIM, TKV], DT, isOutput=False)
    # weights pre-permuted on host to [partition, slice, dc, 512] so every
    # DMA line is one contiguous 8KB-per-partition segment
    wT = nc.declare_dram_parameter("wT", [P, 6, NDC, 512], DT, isOutput=False)
    woT = nc.declare_dram_parameter("woT", [P, 2, NDC, 512], DT, isOutput=False)
    qext = nc.declare_dram_parameter("qext", [4, H, TQ], DT, isOutput=False)
    kext = nc.declare_dram_parameter("kext", [4, H, TKV], DT, isOutput=False)
    gam = nc.declare_dram_parameter("gam", [DH, 2 * H], F32, isOutput=False)
    out = nc.declare_dram_parameter("out", [TQ, DIM], F32, isOutput=True)
    if debug:
        qT_d = nc.declare_dram_parameter("qT_d", [P, NDC, TQ], DT, isOutput=True)
        kT_d = nc.declare_dram_parameter("kT_d", [P, NDC, TKV], DT, isOutput=True)
        V_d = nc.declare_dram_parameter("V_d", [P, NKT, H, DH + 1], DT, isOutput=True)
        aT_d = nc.declare_dram_parameter("aT_d", [P, NDC, TQ], DT, isOutput=True)
        pt_d = nc.declare_dram_parameter("pt_d", [2, P, 2, HW_HALF], DT, isOutput=True)

    with _SplitDrainTileContext(nc) as tc, \
            tc.tile_pool(name="persist", bufs=1) as pp, \
            tc.tile_pool(name="small", bufs=1) as psm:

        # per-head score operands: rows 0..63 = head dims, rows 64..67 = the
        # ALiBi rank-4 rows (folded into the same 68-partition contraction)
        qT = pp.tile([P, H, TQ], DT, tag="qT")         # [p, h, tok]
        kT = pp.tile([P, H, TKV], DT, tag="kT")
        V = pp.tile([P, NKT, H, DH + 1], DT, tag="V")  # [p=tok, kt, h, dh+ones]
        attnT = pp.tile([P, NDC, TQ], DT, tag="attnT")
        gam_sb = pp.tile([DH, 2 * H], F32, tag="gam")
        nc.sync.dma_start(gam_sb[:], gam[:])
        ones_sb = pp.tile([P, 1], F32R, tag="ones")
        ones_row = pp.tile([1, P], F32, tag="onesrow")
        negc_sb = pp.tile([P, 1], F32, tag="negc")
        eps_sb = pp.tile([1, 1], F32, tag="eps")
        ln8_sb = pp.tile([1, 1], F32, tag="ln8")
        # per-pair selection matrices for the reciprocal broadcast matmul:
        # sel[j][g, p] = 1 iff g == 2j + p//64   (K=8 stationary, bf16)
        sel_sb = pp.tile([8, 4, P], BF16, tag="sel")
        nc.vector.memset(ones_sb[:].bitcast(F32), 1.0)
        nc.vector.memset(ones_row[:], 1.0)
        nc.vector.memset(negc_sb[:], -EXP_C)
        nc.vector.memset(eps_sb[:], 1.0e-6)
        nc.vector.memset(ln8_sb[:], float(-0.5 * np.log(64.0)))
        nc.gpsimd.memset(sel_sb[:], 0.0)
        for j in range(4):
            nc.gpsimd.affine_select(
                out=sel_sb[:, j, :], in_=sel_sb[:, j, :],
                compare_op=ALU.not_equal, fill=1.0,
                base=-2 * j, pattern=[[-1, 2], [0, 64]], channel_multiplier=1)
        nonce_sb = pp.tile([1, _src_nonce()], F32, tag="nonce")
        nc.sync.dma_start(nonce_sb[:], nonce[:])
        nc.sync.dma_start(qT[64:68], qext[:])
        nc.sync.dma_start(kT[64:68], kext[:])
        ones_col = V[:, :, :, DH]
        nc.vector.memset(
            ones_col if DT == BF16 else ones_col.bitcast(F32), 1.0)

        # ---------------- Phase A: projections + RMSNorm ----------------
        with tc.tile_pool(name="xp", bufs=1) as px, \
                tc.tile_pool(name="wp", bufs=2) as pw, \
                tc.tile_pool(name="sqp", bufs=2) as psq, \
                tc.tile_pool(name="accp", bufs=1) as pacc, \
                tc.tile_pool(name="psA", bufs=4, space="PSUM") as psA, \
                tc.tile_pool(name="psS1", bufs=2, space="PSUM") as psS1, \
                tc.tile_pool(name="psBC", bufs=2, space="PSUM") as psBC:

            def _dma_w(dst, idx):
                # one contiguous 8KB-per-partition segment, sprayed across
                # 8 queues as 16-partition groups
                for g in range(8):
                    nc.sync.dma_start(dst[16 * g:16 * (g + 1)],
                                      wT[16 * g:16 * (g + 1), idx])

            x_sb = px.tile([P, NDC, TKV], DT, tag="x")
            wq0_sb = pw.tile([P, NDC, 512], DT, tag="wslice", name="wq0")
            _dma_w(wq0_sb, 0)
            for dc in range(NDC):
                for g2 in range(2):
                    nc.sync.dma_start(
                        x_sb[64 * g2:64 * (g2 + 1), dc, :],
                        xT[dc * P + 64 * g2: dc * P + 64 * (g2 + 1), :])

            def _rms_reduce(proj, doff, acc):
                # rsqrt(mean+eps) = exp(-0.5*ln(ss/DIM + eps)); the 1/sqrt(dh)
                # score scale folds into the exp bias for q
                ss = psS1.tile([1, 512], F32, tag="ssq", name="ssq")
                nc.tensor.matmul(ss[:], ones_sb[:].bitcast(F32), acc[:],
                                 start=True, stop=True)
                a = psm.tile([1, 512], F32, tag="a")
                nc.scalar.activation(a[:], ss[:], AF.Ln,
                                     bias=eps_sb[:], scale=1.0 / DIM)
                y = psm.tile([1, 512], F32, tag="y")
                nc.scalar.activation(y[:], a[:], AF.Exp,
                                     bias=(ln8_sb[:] if proj == 0 else 0.0),
                                     scale=-0.5)
                return y

            def _rms_bcast(y):
                # broadcast over partitions via K=1 ones-matmul (plain fp32)
                bc = psBC.tile([P, 512], F32, tag="bc", name="bc")
                nc.tensor.matmul(bc[:], ones_row[:], y[:], start=True, stop=True)
                return bc

            def _normalize(proj, doffs, bcs):
                dst = qT if proj == 0 else kT
                for odt in range(NDC):
                    for hi in range(2):
                        h = 2 * odt + hi
                        gap = gam_sb[:, proj * H + h: proj * H + h + 1]
                        for doff in doffs:
                            sl = dst[0:DH, h, doff:doff + 512]
                            nc.vector.scalar_tensor_tensor(
                                sl, sl, gap, bcs[doff][0:DH, :],
                                op0=ALU.mult, op1=ALU.mult,
                            )

            # Q (tokens 512..1023 of the kv range) and K (all tokens);
            # K group (512,512) first so it only needs the priority x half
            sqacc = {}
            for proj in range(2):
                dst = qT if proj == 0 else kT
                groups = [(TKV - TQ, 0)] if proj == 0 else [(512, 512), (0, 0)]
                for wh in range(2):
                    if proj == 0 and wh == 0:
                        w_sb = wq0_sb
                    else:
                        w_sb = pw.tile([P, NDC, 512], DT, tag="wslice")
                        _dma_w(w_sb, proj * 2 + wh)
                    for ol in range(4):
                        odt = wh * 4 + ol
                        for (soff, doff) in groups:
                            ps = psA.tile([P, 512], F32, tag="projps")
                            for dc in range(NDC):
                                nc.tensor.matmul(
                                    ps[:],
                                    w_sb[:, dc, ol * P:(ol + 1) * P],
                                    x_sb[:, dc, soff:soff + 512],
                                    start=(dc == 0), stop=(dc == NDC - 1),
                                )
                            for hi in range(2):
                                nc.scalar.copy(
                                    dst[0:DH, 2 * odt + hi, doff:doff + 512],
                                    ps[DH * hi:DH * (hi + 1), :])
                            sq = psq.tile([P, 512], F32, tag="sq")
                            nc.scalar.activation(sq[:], ps[:], AF.Square)
                            key = (proj, doff)
                            if odt == 0:
                                acc = pacc.tile([P, 512], F32,
                                                tag=f"acc{proj}_{doff}",
                                                name="acc")
                                sqacc[key] = acc
                                nc.vector.tensor_copy(acc[:], sq[:])
                            else:
                                nc.vector.tensor_add(sqacc[key][:],
                                                     sqacc[key][:], sq[:])
                if proj == 0:
                    # Q's rms chain + normalize can complete during K proj
                    y_q = _rms_reduce(0, 0, sqacc[(0, 0)])
                    _normalize(0, [0], {0: _rms_bcast(y_q)})

            # V projection: [tok, head, dh]; vh innermost so consecutive
            # matmuls share the x-chunk stationary operand; K's rms chain is
            # interleaved so its PE/ACT legs hide behind the V matmuls and
            # the scores can start right at the end of phase A
            wv_sb = []
            for vh in range(2):
                w_sb = pw.tile([P, NDC, 512], DT, tag="wslice")
                _dma_w(w_sb, 4 + vh)
                wv_sb.append(w_sb)
            y_k = {}
            for vt, tt in enumerate((4, 5, 6, 7, 0, 1, 2, 3)):
                pss = [psA.tile([P, 512], F32, tag="projps", name="psv")
                       for _ in range(2)]
                for dc in range(NDC):
                    for vh in range(2):
                        nc.tensor.matmul(
                            pss[vh][:],
                            x_sb[:, dc, tt * P:(tt + 1) * P],
                            wv_sb[vh][:, dc, :],
                            start=(dc == 0), stop=(dc == NDC - 1),
                        )
                for vh in range(2):
                    nc.scalar.copy(
                        V[:, tt, vh * 8:(vh + 1) * 8, :DH],
                        pss[vh][:].rearrange("p (h c) -> p h c", c=DH),
                    )
                if vt == 1:
                    y_k[0] = _rms_reduce(1, 0, sqacc[(1, 0)])
                    y_k[512] = _rms_reduce(1, 512, sqacc[(1, 512)])
            _normalize(1, [0, 512],
                       {d: _rms_bcast(y_k[d]) for d in (0, 512)})

        if debug:
            nc.sync.dma_start(qT_d[:], qT[:])
            nc.sync.dma_start(kT_d[:], kT[:])
            nc.sync.dma_start(V_d[:], V[:])

        # wo prefetch: issued now so the DMAs drain during phase B
        pwo = tc.alloc_tile_pool(name="wop", bufs=2)
        wo_sbs = []
        for oh in range(2):
            w_sb = pwo.tile([P, NDC, 512], DT, tag="wo")
            for g in range(8):
                nc.sync.dma_start(w_sb[16 * g:16 * (g + 1)],
                                  woT[16 * g:16 * (g + 1), oh])
            wo_sbs.append(w_sb)

        # ---------------- Phase B: attention (head pairs) ----------------
        with tc.tile_pool(name="maskp", bufs=1) as pm, \
                tc.tile_pool(name="ptp", bufs=2) as ppt, \
                tc.tile_pool(name="rnp", bufs=1) as prn, \
                tc.tile_pool(name="psS", bufs=1, space="PSUM") as psS, \
                tc.tile_pool(name="psO", bufs=1, space="PSUM") as psO:

            s16 = [pm.tile([8, TQ], F32, tag="s16a", name="s16a"),
                   pm.tile([8, TQ], F32, tag="s16b", name="s16b")]

            for hp in range(NDC):
                ps_o = [psO.tile([DH + 1, TQ], F32, tag=f"avps{hi}", name=f"avps{hi}")
                        for hi in range(2)]
                pts = {0: [], 1: []}
                for half in range(2):
                    ps_pair = [psS.tile([P, HW_HALF], F32, tag=f"sps{hi}", name=f"sps{hi}")
                               for hi in range(2)]
                    for ktl in range(4):
                        kt = half * 4 + ktl
                        off, wdt, qlo = KT_OFF[kt], KT_W[kt], KT_QLO[kt]
                        for hi in range(2):
                            h = 2 * hp + hi
                            # 68-row contraction: head dims + ALiBi rank-4
                            nc.tensor.matmul(
                                ps_pair[hi][:, off:off + wdt],
                                kT[0:DH + 4, h, kt * P:(kt + 1) * P],
                                qT[0:DH + 4, h, qlo:qlo + wdt],
                                start=True, stop=True,
                            )
                    for hi in range(2):
                        pt = ppt.tile([P, HW_HALF], DT, tag=f"pt{hi}")
                        nc.scalar.activation(pt[:], ps_pair[hi][:], AF.Exp,
                                             bias=negc_sb[:])
                        _pt_fixups(nc, pt, half)
                        if debug and hp == 0:
                            nc.sync.dma_start(pt_d[hi, :, half, :], pt[:])
                        pts[hi].append(pt)

                for hi in range(2):
                    h = 2 * hp + hi
                    po = DH * hi
                    # middle key tiles cover all 512 queries in one matmul;
                    # edge tiles touch only their 256-query block
                    for i, (kt, q0, w) in enumerate(AV_PLAN):
                        c0 = KT_OFF[kt] + q0 - KT_QLO[kt]
                        nc.tensor.matmul(
                            ps_o[hi][:, q0:q0 + w],
                            V[:, kt, h, :],
                            pts[hi][kt // 4][:, c0:c0 + w],
                            start=(i == 0), stop=(i == len(AV_PLAN) - 1),
                            skip_group_check=True,
                        )
                    # stash the softmax denominator row; head h -> partition
                    # h of s16 via a tiny SBUF->SBUF DMA (engines cannot write
                    # partition offsets other than 0/32/64)
                    r = prn.tile([1, TQ], F32, tag=f"r{hi}", name="r")
                    nc.vector.tensor_copy(r[:], ps_o[hi][DH:DH + 1, :])
                    nc.sync.dma_start(s16[h // 8][h % 8:h % 8 + 1, :], r[:])
                    nc.vector.tensor_copy(attnT[po:po + DH, hp, :], ps_o[hi][:DH, :])

                if hp % 4 == 3:
                    # normalize the finished half (8 heads): batched reciprocal
                    # on 8 partitions, K=8 selection-matrix matmul broadcasts
                    # each pair's rows across its 128 partitions
                    g = hp // 4
                    rc8 = pm.tile([8, TQ], F32, tag=f"rc{g}", name="rc8")
                    nc.vector.reciprocal(rc8[:], s16[g][:])
                    rc8b = pm.tile([8, TQ], BF16, tag=f"rcb{g}", name="rc8b")
                    nc.vector.tensor_copy(rc8b[:], rc8[:])
                    for hp2 in range(4 * g, 4 * g + 4):
                        j = hp2 % 4
                        rb = psO.tile([P, TQ], F32,
                                      tag=f"avps{hp2 % 2}", name="rb")
                        nc.tensor.matmul(
                            rb[:], sel_sb[:, j, :], rc8b[:],
                            start=True, stop=True)
                        nc.vector.tensor_mul(attnT[:, hp2, :],
                                             attnT[:, hp2, :], rb[:])

        if debug:
            nc.sync.dma_start(aT_d[:], attnT[:])

        # ---------------- Phase C: output projection ----------------
        with tc.tile_pool(name="outp", bufs=1) as pout, \
                tc.tile_pool(name="psC", bufs=3, space="PSUM") as psC:
            out_sb = pout.tile([P, NQT, DIM], F32, tag="out")
            out_r = out.rearrange("(tt p) o -> p tt o", p=P)
            for tt in range(NQT):
                pss = [psC.tile([P, 512], F32, tag="cps", name="psc")
                       for _ in range(2)]
                for adc in range(NDC):
                    for oh in range(2):
                        nc.tensor.matmul(
                            pss[oh][:],
                            attnT[:, adc, tt * P:(tt + 1) * P],
                            wo_sbs[oh][:, adc, :],
                            start=(adc == 0), stop=(adc == NDC - 1),
                        )
                for oh in range(2):
                    nc.vector.tensor_copy(
                        out_sb[:, tt, oh * 512:(oh + 1) * 512], pss[oh][:])
                nc.sync.dma_start(out_r[:, tt, :], out_sb[:, tt, :])
        pwo.release()

    return nc


def _np_dt():
    return np.float32 if PREC == "fp32r" else ml_dtypes.bfloat16


def _build_ext(chunk0: bool):
    """qext [4,H,TQ]: (hi_q, lo_q, 1, 1);  kext [4,H,TKV]: (1, 1, k_hi, k_lo).
    hi/lo are a two-term bf16 split of -slope_h*(q_local+512); k_hi/k_lo of
    slope_h*k_local. Padded halo keys (chunk 0) get k_hi = -3e4 -> exp = 0."""
    np_dt = _np_dt()
    qe = np.zeros((4, H, TQ), np.float32)
    ke = np.zeros((4, H, TKV), np.float32)
    qpos = np.arange(TQ, dtype=np.float64) + 512.0
    kpos = np.arange(TKV, dtype=np.float64)
    qe[2:] = 1.0
    ke[:2] = 1.0
    qhi = np.zeros((H, TQ), np_dt)
    qlo = np.zeros((H, TQ), np_dt)
    khi = np.zeros((H, TKV), np_dt)
    klo = np.zeros((H, TKV), np_dt)
    for h in range(H):
        tq = (-SLOPES[h] * qpos).astype(np.float32)
        hi = tq.astype(np_dt)
        qhi[h] = hi
        qlo[h] = (tq - hi.astype(np.float32)).astype(np_dt)
        tk = (SLOPES[h] * kpos).astype(np.float32)
        if chunk0:
            tk[:512] = -3.0e4
        hi = tk.astype(np_dt)
        khi[h] = hi
        lo = (tk - hi.astype(np.float32))
        if chunk0:
            lo[:512] = 0.0
        klo[h] = lo.astype(np_dt)
    qe = qe.astype(np_dt)
    ke = ke.astype(np_dt)
    qe[0], qe[1] = qhi, qlo
    ke[2], ke[3] = khi, klo
    return np.ascontiguousarray(qe), np.ascontiguousarray(ke)


_NC = None
LAST = None  # BassKernelResults of the most recent run (exec_time_ns when traced)


def _get_nc():
    global _NC
    if _NC is None:
        _NC = _build_program()
    return _NC


def kernel(x, wq, wk, wv, wo, q_gamma, k_gamma):
    x = np.ascontiguousarray(np.asarray(x, np.float32))
    wq = np.asarray(wq, np.float32)
    wk = np.asarray(wk, np.float32)
    wv = np.asarray(wv, np.float32)
    wo = np.asarray(wo, np.float32)
    q_gamma = np.asarray(q_gamma, np.float32)
    k_gamma = np.asarray(k_gamma, np.float32)

    np_dt = _np_dt()
    # [in, out] -> [p, slice, dc, o]: per-partition-contiguous 8KB DMA lines
    wcat = np.concatenate([wq.T, wk.T, wv.T], axis=1)
    wT_host = np.ascontiguousarray(
        wcat.reshape(NDC, P, 6, 512).transpose(1, 2, 0, 3).astype(np_dt))
    woT_host = np.ascontiguousarray(
        wo.T.reshape(NDC, P, 2, 512).transpose(1, 2, 0, 3).astype(np_dt))
    gam_host = np.ascontiguousarray(np.concatenate(
        [q_gamma.reshape(H, DH).T, k_gamma.reshape(H, DH).T], axis=1))
    qe0, ke0 = _build_ext(True)
    qei, kei = _build_ext(False)

    in_maps = []
    for c in range(NCORES):
        b, j = divmod(c, 4)
        lo = j * TQ - WINDOW
        xs = x[b, max(0, lo): j * TQ + TQ, :]
        if lo < 0:
            xs = np.concatenate(
                [np.zeros((-lo, DIM), np.float32), xs], axis=0)
        in_maps.append({
            "nonce": np.zeros((1, _src_nonce()), np.float32),
            "xT": np.ascontiguousarray(xs.T.astype(np_dt)),
            "wT": wT_host,
            "woT": woT_host,
            "gam": gam_host,
            "qext": qe0 if j == 0 else qei,
            "kext": ke0 if j == 0 else kei,
        })

    global LAST
    trace = bool(int(os.environ.get("KERNEL_TRACE", "0") or 0))
    try:
        LAST = run_bass_kernel_spmd(
            _get_nc(), in_maps, list(range(NCORES)), trace=trace)
    except Exception:
        # a previously-wedged device surfaces as NRT_EXEC_UNIT_UNRECOVERABLE
        # on the first touch; reset the accelerator once and retry
        try:
            lib = ctypes.CDLL("/opt/axon/libaxon_pjrt.so")
            lib.axon_reset.restype = ctypes.c_int64
            import jax
            jax.devices()
            lib.axon_reset()
        except Exception:
            pass
        LAST = run_bass_kernel_spmd(
            _get_nc(), in_maps, list(range(NCORES)), trace=trace)

    full = np.empty((B, T, DIM), np.float32)
    for c in range(NCORES):
        b, j = divmod(c, 4)
        full[b, j * TQ:(j + 1) * TQ, :] = LAST.results[c]["out"]
    return full


# revision 39
# speedup vs baseline: 1.3621x; 1.0904x over previous
"""CodecAttention (sliding-window attention w/ QK-RMSNorm + ALiBi) on 8 trn2 cores.

Sharding: data-parallel over (batch, sequence-chunk): 2 batches x 4 chunks of 512
queries -> 8 cores. Each core recomputes K/V for its 512-token halo (zero-padded
for the first chunk), so there is no cross-core communication; the host only
slices/transposes inputs and concatenates the 8 disjoint output slices.

On-core pipeline (bf16 operands, fp32 PSUM accumulation):
  A) QKV projections from x^T (dim-on-partitions); QK RMSNorm over the flat
     1024-dim axis via ACT-square + accumulating ones-matmul partition
     reduction, rsqrt as exp(-0.5*ln(x)), 1/sqrt(dh) folded into the q scale.
  B) Head pairs: S^T tiles = k^T.T @ q^T as a 68-row contraction per head
     (64 head dims + 4 ALiBi rows: hi/lo bf16-split position terms; padded
     halo keys of chunk 0 killed via a -3e4 key bias), ACT exp
     with a fixed offset straight off PSUM (scores bounded), gpsimd
     affine_select/memset zero the causal/window boundary triangles and
     dead rectangles on the post-exp pt tiles, AV+rowsum with V-as-stationary
     (ones column yields softmax denominators), denominators DMA'd from PSUM
     into an [8, TQ] tile, one grouped reciprocal, K=8 selection-matrix
     matmul broadcasts the reciprocals per pair, per-pair normalize multiply.
  C) out = attnT.T @ wo^T per token tile, streamed 512KB output DMAs.
"""

import contextlib
import ctypes
import os
import sys
import types

import ml_dtypes
import numpy as np

import concourse.bass as bass
import concourse.mybir as mybir
import concourse.tile as tile


def _install_axon_ntff_shim():
    """bass_utils' trace path wants antenv.axon_hooks, which this image lacks.
    Provide it, backed by direct ctypes calls into libaxon_pjrt.so (same ABI
    the agent boot would use). Degrades to hook=None if the .so is absent."""
    try:
        import antenv.axon_hooks  # noqa: F401
        return
    except ImportError:
        pass

    _hook_holder = [None]
    so_path = "/opt/axon/libaxon_pjrt.so"
    if os.path.exists(so_path):
        try:
            lib = ctypes.CDLL(so_path)
            if hasattr(lib, "axon_start_nrt_profile"):
                lib.axon_start_nrt_profile.argtypes = [
                    ctypes.POINTER(ctypes.c_int64), ctypes.c_size_t]
                lib.axon_start_nrt_profile.restype = ctypes.c_int64
                lib.axon_stop_nrt_profile.argtypes = [ctypes.c_char_p]
                lib.axon_stop_nrt_profile.restype = ctypes.c_int64

                @contextlib.contextmanager
                def _hook(output_dir, device_ids):
                    import jax
                    jax.devices()
                    if device_ids:
                        ids = (ctypes.c_int64 * len(device_ids))(*device_ids)
                        rc = lib.axon_start_nrt_profile(ids, len(device_ids))
                    else:
                        rc = lib.axon_start_nrt_profile(None, 0)
                    if rc != 0:
                        raise RuntimeError(f"axon_start_nrt_profile rc={rc}")
                    try:
                        yield
                    finally:
                        n = lib.axon_stop_nrt_profile(str(output_dir).encode())
                        if n < 0:
                            raise RuntimeError(f"axon_stop_nrt_profile rc={n}")

                _hook_holder[0] = _hook
        except OSError:
            pass

    mod = types.ModuleType("antenv.axon_hooks")
    mod.get_axon_ntff_profile_hook = lambda: _hook_holder[0]
    mod.set_axon_ntff_profile_hook = lambda h: _hook_holder.__setitem__(0, h)
    sys.modules["antenv.axon_hooks"] = mod


_install_axon_ntff_shim()

from concourse.bass_utils import run_bass_kernel_spmd  # noqa: E402
from bass_rust import ScopedClock  # noqa: E402

B, T, DIM = 2, 2048, 1024
H, DH, WINDOW = 16, 64, 512
P = 128
TQ = 512            # queries per core
TKV = 1024          # kv tokens per core (incl. 512 halo)
NCORES = 8
NQT = TQ // P       # 4
NKT = TKV // P      # 8
NDC = DIM // P      # 8
EXP_C = 10.0        # exp offset; true max masked score is ~6.0 for this data
F32 = mybir.dt.float32
F32R = mybir.dt.float32r
BF16 = mybir.dt.bfloat16
PREC = os.environ.get("KERNEL_PREC", "bf16")
DT = F32R if PREC == "fp32r" else BF16
AF = mybir.ActivationFunctionType
ALU = mybir.AluOpType

SLOPES = [2.0 ** (-0.5 * (h + 1)) for h in range(H)]

# Score-tile packing: per head, scores are computed as S^T [key, query] in two
# 3-bank PSUM halves of [128, 1536]. Key-tile kt covers queries
# [KT_QLO[kt], KT_QLO[kt]+KT_W[kt]) at column offset KT_OFF[kt] of its half.
KT_W = [256, 256, 512, 512, 512, 512, 256, 256]
KT_OFF = [0, 256, 512, 1024, 0, 512, 1024, 1280]
KT_QLO = [0, 0, 0, 0, 0, 0, 256, 256]
HW_HALF = 1536
# post-exp boundary cleanup per half: (col0, kind) with kind 'win' (keep
# key>=query within the block), 'causal' (keep query>=key), 'dead' (zero all).
# Block (kt, qb) is valid iff qb <= kt <= qb+4; kt==qb is the window edge,
# kt==qb+4 the causal edge; anything else computed by the packing is dead.
PT_FIX = {
    0: [(0, "win"), (128, "dead"), (256 + 128, "win"),
        (512 + 256, "win"), (512 + 384, "dead"), (1024 + 384, "win")],
    1: [(0, "causal"), (512, "dead"), (512 + 128, "causal"),
        (1024, "causal"), (1280, "dead"), (1280 + 128, "causal")],
}
# merged AV matmul plan: (kt, out_col_lo, width); the four middle key tiles
# serve all 512 queries in one 512-wide matmul, edge tiles only their 256-
# query block (per-element PSUM has_written handles the region accumulation;
# the first, full-width matmul initializes every element)
AV_PLAN = [
    (2, 0, 512), (3, 0, 512), (4, 0, 512), (5, 0, 512),
    (0, 0, 256), (1, 0, 256), (6, 256, 256), (7, 256, 256),
]


class _SplitDrainTileContext(tile.TileContext):
    """The walrus build in this env rejects >1-2 sync-wait commands on one
    instruction; spread excess waits across same-engine NOPs placed directly
    before the over-limit instruction (per-engine program order preserved)."""

    def _split_excess_waits(self):
        nc = self.nc
        cur_list = nc.cur_bb.bb.instructions
        for blk in nc.m.functions[0].blocks:
            snapshot = list(blk.instructions)
            for inst in snapshot:
                si = inst.sync_info
                max_w = 1
                if si is None or len(si.on_wait) <= max_w:
                    continue
                waits = list(si.on_wait)
                si.on_wait = waits[:max_w]
                eng_obj = nc.engines[inst.engine]
                for w in waits[max_w:]:
                    nop_bi = eng_obj.nop(nofuse=True, hint="wait_split")
                    nop_inst = nop_bi.ins
                    nop_inst.sync_info = mybir.SyncInfo(on_wait=[w], on_update=[])
                    cur_list.remove(nop_inst)
                    blk.instructions.insert(
                        blk.instructions.index(inst), nop_inst)

    def _drain_and_barrier(self, tick_clock, wait_clock):
        self._split_excess_waits()
        drain_inst = self.nc.sync.drain()
        wait_clock.add_sem_waits(
            drain_inst.ins, ScopedClock({None: tick_clock.global_clock})
        )
        si = drain_inst.ins.sync_info
        if si is not None and len(si.on_wait) > 1:
            waits = list(si.on_wait)
            si.on_wait = waits[:1]
            for w in waits[1:]:
                nop = self.nc.sync.nop(nofuse=True, hint="drain_wait_split")
                nop.ins.sync_info = mybir.SyncInfo(on_wait=[w], on_update=[])
        self.nc.all_engine_barrier()
        assert self.sems is not None
        popped = self.nc._tile_sem_poison_stack.pop()
        assert popped is self._sem_poison
        self.nc.clear_and_free_semaphores(list(self.sems.allocated().values()))
        self.nc.all_engine_barrier()


def _src_nonce():
    import zlib
    with open(__file__, "rb") as f:
        return (zlib.crc32(f.read() + PREC.encode()) % 2048) + 8


def _pt_fixups(nc, pt, half):
    """Zero the invalid regions of a post-exp pt tile on the gpsimd engine."""
    for col0, kind in PT_FIX[half]:
        sl = pt[:, col0:col0 + P]
        if kind == "dead":
            nc.gpsimd.memset(sl, 0.0)
        elif kind == "win":
            # keep iff key_local >= query_local  (iota = p - qi >= 0)
            nc.gpsimd.affine_select(
                out=sl, in_=sl, compare_op=ALU.is_ge, fill=0.0,
                base=0, pattern=[[-1, P]], channel_multiplier=1)
        else:
            # causal: keep iff query_local >= key_local (iota = qi - p >= 0)
            nc.gpsimd.affine_select(
                out=sl, in_=sl, compare_op=ALU.is_ge, fill=0.0,
                base=0, pattern=[[1, P]], channel_multiplier=-1)


def _build_program(debug=False):
    nc = bass.Bass()
    # dummy input whose shape changes with this file: busts HLO-keyed NEFF
    # caches (the BIR itself is not part of the HLO fingerprint)
    nonce = nc.declare_dram_parameter("nonce", [1, _src_nonce()], F32,
                                      isOutput=False)
    xT = nc.declare_dram_parameter("xT", [DIM, TKV], DT, isOutput=False)
    # weights pre-permuted on host to [partition, slice, dc, 512] so every
    # DMA line is one contiguous 8KB-per-partition segment
    wT = nc.declare_dram_parameter("wT", [P, 6, NDC, 512], DT, isOutput=False)
    woT = nc.declare_dram_parameter("woT", [P, 2, NDC, 512], DT, isOutput=False)
    qext = nc.declare_dram_parameter("qext", [4, H, TQ], DT, isOutput=False)
    kext = nc.declare_dram_parameter("kext", [4, H, TKV], DT, isOutput=False)
    gam = nc.declare_dram_parameter("gam", [DH, 2 * H], F32, isOutput=False)
    out = nc.declare_dram_parameter("out", [TQ, DIM], F32, isOutput=True)
    if debug:
        qT_d = nc.declare_dram_parameter("qT_d", [P, NDC, TQ], DT, isOutput=True)
        kT_d = nc.declare_dram_parameter("kT_d", [P, NDC, TKV], DT, isOutput=True)
        V_d = nc.declare_dram_parameter("V_d", [P, NKT, H, DH + 1], DT, isOutput=True)
        aT_d = nc.declare_dram_parameter("aT_d", [P, NDC, TQ], DT, isOutput=True)
        pt_d = nc.declare_dram_parameter("pt_d", [2, P, 2, HW_HALF], DT, isOutput=True)

    with _SplitDrainTileContext(nc) as tc, \
            tc.tile_pool(name="persist", bufs=1) as pp, \
            tc.tile_pool(name="small", bufs=1) as psm:

        # per-head score operands: rows 0..63 = head dims, rows 64..67 = the
        # ALiBi rank-4 rows (folded into the same 68-partition contraction)
        qT = pp.tile([P, H, TQ], DT, tag="qT")         # [p, h, tok]
        kT = pp.tile([P, H, TKV], DT, tag="kT")
        V = pp.tile([P, NKT, H, DH + 1], DT, tag="V")  # [p=tok, kt, h, dh+ones]
        attnT = pp.tile([P, NDC, TQ], DT, tag="attnT")
        gam_sb = pp.tile([DH, 2 * H], F32, tag="gam")
        nc.sync.dma_start(gam_sb[:], gam[:])
        ones_sb = pp.tile([P, 1], F32R, tag="ones")
        ones_row = pp.tile([1, P], F32, tag="onesrow")
        negc_sb = pp.tile([P, 1], F32, tag="negc")
        eps_sb = pp.tile([1, 1], F32, tag="eps")
        ln8_sb = pp.tile([1, 1], F32, tag="ln8")
        # per-pair selection matrices for the reciprocal broadcast matmul:
        # sel[j][g, p] = 1 iff g == 2j + p//64   (K=8 stationary, bf16)
        sel_sb = pp.tile([8, 4, P], BF16, tag="sel")
        nc.vector.memset(ones_sb[:].bitcast(F32), 1.0)
        nc.vector.memset(ones_row[:], 1.0)
        nc.vector.memset(negc_sb[:], -EXP_C)
        nc.vector.memset(eps_sb[:], 1.0e-6)
        nc.vector.memset(ln8_sb[:], float(-0.5 * np.log(64.0)))
        nc.gpsimd.memset(sel_sb[:], 0.0)
        for j in range(4):
            nc.gpsimd.affine_select(
                out=sel_sb[:, j, :], in_=sel_sb[:, j, :],
                compare_op=ALU.not_equal, fill=1.0,
                base=-2 * j, pattern=[[-1, 2], [0, 64]], channel_multiplier=1)
        nonce_sb = pp.tile([1, _src_nonce()], F32, tag="nonce")
        nc.sync.dma_start(nonce_sb[:], nonce[:])
        nc.sync.dma_start(qT[64:68], qext[:])
        nc.sync.dma_start(kT[64:68], kext[:])
        ones_col = V[:, :, :, DH]
        nc.vector.memset(
            ones_col if DT == BF16 else ones_col.bitcast(F32), 1.0)

        # ---------------- Phase A: projections + RMSNorm ----------------
        with tc.tile_pool(name="xp", bufs=1) as px, \
                tc.tile_pool(name="wp", bufs=2) as pw, \
                tc.tile_pool(name="sqp", bufs=2) as psq, \
                tc.tile_pool(name="accp", bufs=1) as pacc, \
                tc.tile_pool(name="psA", bufs=4, space="PSUM") as psA, \
                tc.tile_pool(name="psS1", bufs=2, space="PSUM") as psS1, \
                tc.tile_pool(name="psBC", bufs=2, space="PSUM") as psBC:

            def _dma_w(dst, idx):
                # dc-pair chunks: full 128 partitions, 2KB contiguous lines
                for g in range(4):
                    nc.sync.dma_start(dst[:, 2 * g:2 * (g + 1), :],
                                      wT[:, idx, 2 * g:2 * (g + 1), :])

            x_sb = px.tile([P, NDC, TKV], DT, tag="x")
            wq0_sb = pw.tile([P, NDC, 512], DT, tag="wslice", name="wq0")
            _dma_w(wq0_sb, 0)
            for dc in range(NDC):
                nc.sync.dma_start(x_sb[:, dc, :],
                                  xT[dc * P:(dc + 1) * P, :])

            def _rms_reduce(proj, doff, acc):
                # rsqrt(mean+eps) = exp(-0.5*ln(ss/DIM + eps)); the 1/sqrt(dh)
                # score scale folds into the exp bias for q
                ss = psS1.tile([1, 512], F32, tag="ssq", name="ssq")
                nc.tensor.matmul(ss[:], ones_sb[:].bitcast(F32), acc[:],
                                 start=True, stop=True)
                a = psm.tile([1, 512], F32, tag="a")
                nc.scalar.activation(a[:], ss[:], AF.Ln,
                                     bias=eps_sb[:], scale=1.0 / DIM)
                y = psm.tile([1, 512], F32, tag="y")
                nc.scalar.activation(y[:], a[:], AF.Exp,
                                     bias=(ln8_sb[:] if proj == 0 else 0.0),
                                     scale=-0.5)
                return y

            def _rms_bcast(y):
                # broadcast over partitions via K=1 ones-matmul (plain fp32)
                bc = psBC.tile([P, 512], F32, tag="bc", name="bc")
                nc.tensor.matmul(bc[:], ones_row[:], y[:], start=True, stop=True)
                return bc

            def _normalize(proj, doffs, bcs):
                dst = qT if proj == 0 else kT
                for odt in range(NDC):
                    for hi in range(2):
                        h = 2 * odt + hi
                        gap = gam_sb[:, proj * H + h: proj * H + h + 1]
                        for doff in doffs:
                            sl = dst[0:DH, h, doff:doff + 512]
                            nc.vector.scalar_tensor_tensor(
                                sl, sl, gap, bcs[doff][0:DH, :],
                                op0=ALU.mult, op1=ALU.mult,
                            )

            # Q (tokens 512..1023 of the kv range) and K (all tokens);
            # K group (512,512) first so it only needs the priority x half
            sqacc = {}
            for proj in range(2):
                dst = qT if proj == 0 else kT
                groups = [(TKV - TQ, 0)] if proj == 0 else [(512, 512), (0, 0)]
                for wh in range(2):
                    if proj == 0 and wh == 0:
                        w_sb = wq0_sb
                    else:
                        w_sb = pw.tile([P, NDC, 512], DT, tag="wslice")
                        _dma_w(w_sb, proj * 2 + wh)
                    for ol in range(4):
                        odt = wh * 4 + ol
                        for (soff, doff) in groups:
                            ps = psA.tile([P, 512], F32, tag="projps")
                            for dc in range(NDC):
                                nc.tensor.matmul(
                                    ps[:],
                                    w_sb[:, dc, ol * P:(ol + 1) * P],
                                    x_sb[:, dc, soff:soff + 512],
                                    start=(dc == 0), stop=(dc == NDC - 1),
                                )
                            # square first: the rms chain (ACT FIFO) must
                            # not queue behind the eviction copies
                            sq = psq.tile([P, 512], F32, tag="sq")
                            nc.scalar.activation(sq[:], ps[:], AF.Square)
                            for hi in range(2):
                                nc.scalar.copy(
                                    dst[0:DH, 2 * odt + hi, doff:doff + 512],
                                    ps[DH * hi:DH * (hi + 1), :])
                            key = (proj, doff)
                            if odt == 0:
                                acc = pacc.tile([P, 512], F32,
                                                tag=f"acc{proj}_{doff}",
                                                name="acc")
                                sqacc[key] = acc
                                nc.vector.tensor_copy(acc[:], sq[:])
                            else:
                                nc.vector.tensor_add(sqacc[key][:],
                                                     sqacc[key][:], sq[:])
                if proj == 0:
                    # Q's rms chain + normalize can complete during K proj
                    y_q = _rms_reduce(0, 0, sqacc[(0, 0)])
                    _normalize(0, [0], {0: _rms_bcast(y_q)})

            # V projection: [tok, head, dh]; vh innermost so consecutive
            # matmuls share the x-chunk stationary operand; K's rms chain is
            # interleaved so its PE/ACT legs hide behind the V matmuls and
            # the scores can start right at the end of phase A
            wv_sb = []
            for vh in range(2):
                w_sb = pw.tile([P, NDC, 512], DT, tag="wslice")
                _dma_w(w_sb, 4 + vh)
                wv_sb.append(w_sb)
            y_k = {}
            for vt, tt in enumerate((4, 5, 6, 7, 0, 1, 2, 3)):
                pss = [psA.tile([P, 512], F32, tag="projps", name="psv")
                       for _ in range(2)]
                for dc in range(NDC):
                    for vh in range(2):
                        nc.tensor.matmul(
                            pss[vh][:],
                            x_sb[:, dc, tt * P:(tt + 1) * P],
                            wv_sb[vh][:, dc, :],
                            start=(dc == 0), stop=(dc == NDC - 1),
                        )
                for vh in range(2):
                    nc.scalar.copy(
                        V[:, tt, vh * 8:(vh + 1) * 8, :DH],
                        pss[vh][:].rearrange("p (h c) -> p h c", c=DH),
                    )
                if vt == 1:
                    y_k[0] = _rms_reduce(1, 0, sqacc[(1, 0)])
                    y_k[512] = _rms_reduce(1, 512, sqacc[(1, 512)])
            _normalize(1, [0, 512],
                       {d: _rms_bcast(y_k[d]) for d in (0, 512)})

        if debug:
            nc.sync.dma_start(qT_d[:], qT[:])
            nc.sync.dma_start(kT_d[:], kT[:])
            nc.sync.dma_start(V_d[:], V[:])

        # wo prefetch: issued now so the DMAs drain during phase B
        pwo = tc.alloc_tile_pool(name="wop", bufs=2)
        wo_sbs = []
        for oh in range(2):
            w_sb = pwo.tile([P, NDC, 512], DT, tag="wo")
            for g in range(4):
                nc.sync.dma_start(w_sb[:, 2 * g:2 * (g + 1), :],
                                  woT[:, oh, 2 * g:2 * (g + 1), :])
            wo_sbs.append(w_sb)

        # ---------------- Phase B: attention (head pairs) ----------------
        with tc.tile_pool(name="maskp", bufs=1) as pm, \
                tc.tile_pool(name="ptp", bufs=2) as ppt, \
                tc.tile_pool(name="rnp", bufs=1) as prn, \
                tc.tile_pool(name="psS", bufs=1, space="PSUM") as psS, \
                tc.tile_pool(name="psO", bufs=1, space="PSUM") as psO:

            s16 = [pm.tile([8, TQ], F32, tag="s16a", name="s16a"),
                   pm.tile([8, TQ], F32, tag="s16b", name="s16b")]

            for hp in range(NDC):
                ps_o = [psO.tile([DH + 1, TQ], F32, tag=f"avps{hi}", name=f"avps{hi}")
                        for hi in range(2)]
                pts = {0: [], 1: []}
                for half in range(2):
                    ps_pair = [psS.tile([P, HW_HALF], F32, tag=f"sps{hi}", name=f"sps{hi}")
                               for hi in range(2)]
                    for ktl in range(4):
                        kt = half * 4 + ktl
                        off, wdt, qlo = KT_OFF[kt], KT_W[kt], KT_QLO[kt]
                        for hi in range(2):
                            h = 2 * hp + hi
                            # 68-row contraction: head dims + ALiBi rank-4
                            nc.tensor.matmul(
                                ps_pair[hi][:, off:off + wdt],
                                kT[0:DH + 4, h, kt * P:(kt + 1) * P],
                                qT[0:DH + 4, h, qlo:qlo + wdt],
                                start=True, stop=True,
                            )
                    for hi in range(2):
                        pt = ppt.tile([P, HW_HALF], DT, tag=f"pt{hi}")
                        nc.scalar.activation(pt[:], ps_pair[hi][:], AF.Exp,
                                             bias=negc_sb[:])
                        _pt_fixups(nc, pt, half)
                        if debug and hp == 0:
                            nc.sync.dma_start(pt_d[hi, :, half, :], pt[:])
                        pts[hi].append(pt)

                for hi in range(2):
                    h = 2 * hp + hi
                    po = DH * hi
                    # middle key tiles cover all 512 queries in one matmul;
                    # edge tiles touch only their 256-query block
                    for i, (kt, q0, w) in enumerate(AV_PLAN):
                        c0 = KT_OFF[kt] + q0 - KT_QLO[kt]
                        nc.tensor.matmul(
                            ps_o[hi][:, q0:q0 + w],
                            V[:, kt, h, :],
                            pts[hi][kt // 4][:, c0:c0 + w],
                            start=(i == 0), stop=(i == len(AV_PLAN) - 1),
                            skip_group_check=True,
                        )
                    # stash the softmax denominator row; head h -> partition
                    # h of s16 via a tiny SBUF->SBUF DMA (engines cannot write
                    # partition offsets other than 0/32/64)
                    r = prn.tile([1, TQ], F32, tag=f"r{hi}", name="r")
                    nc.vector.tensor_copy(r[:], ps_o[hi][DH:DH + 1, :])
                    nc.sync.dma_start(s16[h // 8][h % 8:h % 8 + 1, :], r[:])
                    nc.vector.tensor_copy(attnT[po:po + DH, hp, :], ps_o[hi][:DH, :])

                if hp % 4 == 3:
                    # normalize the finished half (8 heads): batched reciprocal
                    # on 8 partitions, K=8 selection-matrix matmul broadcasts
                    # each pair's rows across its 128 partitions
                    g = hp // 4
                    rc8 = pm.tile([8, TQ], F32, tag=f"rc{g}", name="rc8")
                    nc.vector.reciprocal(rc8[:], s16[g][:])
                    rc8b = pm.tile([8, TQ], BF16, tag=f"rcb{g}", name="rc8b")
                    nc.vector.tensor_copy(rc8b[:], rc8[:])
                    for hp2 in range(4 * g, 4 * g + 4):
                        j = hp2 % 4
                        rb = psO.tile([P, TQ], F32,
                                      tag=f"avps{hp2 % 2}", name="rb")
                        nc.tensor.matmul(
                            rb[:], sel_sb[:, j, :], rc8b[:],
                            start=True, stop=True)
                        nc.vector.tensor_mul(attnT[:, hp2, :],
                                             attnT[:, hp2, :], rb[:])

        if debug:
            nc.sync.dma_start(aT_d[:], attnT[:])

        # ---------------- Phase C: output projection ----------------
        with tc.tile_pool(name="outp", bufs=1) as pout, \
                tc.tile_pool(name="psC", bufs=3, space="PSUM") as psC:
            out_sb = pout.tile([P, NQT, DIM], F32, tag="out")
            out_r = out.rearrange("(tt p) o -> p tt o", p=P)
            for tt in range(NQT):
                pss = [psC.tile([P, 512], F32, tag="cps", name="psc")
                       for _ in range(2)]
                for adc in range(NDC):
                    for oh in range(2):
                        nc.tensor.matmul(
                            pss[oh][:],
                            attnT[:, adc, tt * P:(tt + 1) * P],
                            wo_sbs[oh][:, adc, :],
                            start=(adc == 0), stop=(adc == NDC - 1),
                        )
                for oh in range(2):
                    nc.vector.tensor_copy(
                        out_sb[:, tt, oh * 512:(oh + 1) * 512], pss[oh][:])
                for g in range(4):
                    nc.sync.dma_start(
                        out_r[32 * g:32 * (g + 1), tt, :],
                        out_sb[32 * g:32 * (g + 1), tt, :])
        pwo.release()

    return nc


def _np_dt():
    return np.float32 if PREC == "fp32r" else ml_dtypes.bfloat16


def _build_ext(chunk0: bool):
    """qext [4,H,TQ]: (hi_q, lo_q, 1, 1);  kext [4,H,TKV]: (1, 1, k_hi, k_lo).
    hi/lo are a two-term bf16 split of -slope_h*(q_local+512); k_hi/k_lo of
    slope_h*k_local. Padded halo keys (chunk 0) get k_hi = -3e4 -> exp = 0."""
    np_dt = _np_dt()
    qe = np.zeros((4, H, TQ), np.float32)
    ke = np.zeros((4, H, TKV), np.float32)
    qpos = np.arange(TQ, dtype=np.float64) + 512.0
    kpos = np.arange(TKV, dtype=np.float64)
    qe[2:] = 1.0
    ke[:2] = 1.0
    qhi = np.zeros((H, TQ), np_dt)
    qlo = np.zeros((H, TQ), np_dt)
    khi = np.zeros((H, TKV), np_dt)
    klo = np.zeros((H, TKV), np_dt)
    for h in range(H):
        tq = (-SLOPES[h] * qpos).astype(np.float32)
        hi = tq.astype(np_dt)
        qhi[h] = hi
        qlo[h] = (tq - hi.astype(np.float32)).astype(np_dt)
        tk = (SLOPES[h] * kpos).astype(np.float32)
        if chunk0:
            tk[:512] = -3.0e4
        hi = tk.astype(np_dt)
        khi[h] = hi
        lo = (tk - hi.astype(np.float32))
        if chunk0:
            lo[:512] = 0.0
        klo[h] = lo.astype(np_dt)
    qe = qe.astype(np_dt)
    ke = ke.astype(np_dt)
    qe[0], qe[1] = qhi, qlo
    ke[2], ke[3] = khi, klo
    return np.ascontiguousarray(qe), np.ascontiguousarray(ke)


_NC = None
LAST = None  # BassKernelResults of the most recent run (exec_time_ns when traced)


def _get_nc():
    global _NC
    if _NC is None:
        _NC = _build_program()
    return _NC


def kernel(x, wq, wk, wv, wo, q_gamma, k_gamma):
    x = np.ascontiguousarray(np.asarray(x, np.float32))
    wq = np.asarray(wq, np.float32)
    wk = np.asarray(wk, np.float32)
    wv = np.asarray(wv, np.float32)
    wo = np.asarray(wo, np.float32)
    q_gamma = np.asarray(q_gamma, np.float32)
    k_gamma = np.asarray(k_gamma, np.float32)

    np_dt = _np_dt()
    # [in, out] -> [p, slice, dc, o]: per-partition-contiguous 8KB DMA lines
    wcat = np.concatenate([wq.T, wk.T, wv.T], axis=1)
    wT_host = np.ascontiguousarray(
        wcat.reshape(NDC, P, 6, 512).transpose(1, 2, 0, 3).astype(np_dt))
    woT_host = np.ascontiguousarray(
        wo.T.reshape(NDC, P, 2, 512).transpose(1, 2, 0, 3).astype(np_dt))
    gam_host = np.ascontiguousarray(np.concatenate(
        [q_gamma.reshape(H, DH).T, k_gamma.reshape(H, DH).T], axis=1))
    qe0, ke0 = _build_ext(True)
    qei, kei = _build_ext(False)

    in_maps = []
    for c in range(NCORES):
        b, j = divmod(c, 4)
        lo = j * TQ - WINDOW
        xs = x[b, max(0, lo): j * TQ + TQ, :]
        if lo < 0:
            xs = np.concatenate(
                [np.zeros((-lo, DIM), np.float32), xs], axis=0)
        in_maps.append({
            "nonce": np.zeros((1, _src_nonce()), np.float32),
            "xT": np.ascontiguousarray(xs.T.astype(np_dt)),
            "wT": wT_host,
            "woT": woT_host,
            "gam": gam_host,
            "qext": qe0 if j == 0 else qei,
            "kext": ke0 if j == 0 else kei,
        })

    global LAST
    trace = bool(int(os.environ.get("KERNEL_TRACE", "0") or 0))
    try:
        LAST = run_bass_kernel_spmd(
            _get_nc(), in_maps, list(range(NCORES)), trace=trace)
    except Exception:
        # a previously-wedged device surfaces as NRT_EXEC_UNIT_UNRECOVERABLE
        # on the first touch; reset the accelerator once and retry
        try:
            lib = ctypes.CDLL("/opt/axon/libaxon_pjrt.so")
            lib.axon_reset.restype = ctypes.c_int64
            import jax
            jax.devices()
            lib.axon_reset()
        except Exception:
            pass
        LAST = run_bass_kernel_spmd(
            _get_nc(), in_maps, list(range(NCORES)), trace=trace)

    full = np.empty((B, T, DIM), np.float32)
    for c in range(NCORES):
        b, j = divmod(c, 4)
        full[b, j * TQ:(j + 1) * TQ, :] = LAST.results[c]["out"]
    return full


# revision 45
# speedup vs baseline: 1.3714x; 1.0068x over previous
"""CodecAttention (sliding-window attention w/ QK-RMSNorm + ALiBi) on 8 trn2 cores.

Sharding: data-parallel over (batch, sequence-chunk): 2 batches x 4 chunks of 512
queries -> 8 cores. Each core recomputes K/V for its 512-token halo (zero-padded
for the first chunk), so there is no cross-core communication; the host only
slices/transposes inputs and concatenates the 8 disjoint output slices.

On-core pipeline (bf16 operands, fp32 PSUM accumulation):
  A) QKV projections from x^T (dim-on-partitions); QK RMSNorm over the flat
     1024-dim axis via ACT-square + accumulating ones-matmul partition
     reduction, rsqrt as exp(-0.5*ln(x)), 1/sqrt(dh) folded into the q scale.
  B) Head pairs: S^T tiles = k^T.T @ q^T as a 68-row contraction per head
     (64 head dims + 4 ALiBi rows: hi/lo bf16-split position terms; padded
     halo keys of chunk 0 killed via a -3e4 key bias), ACT exp
     with a fixed offset straight off PSUM (scores bounded), gpsimd
     affine_select/memset zero the causal/window boundary triangles and
     dead rectangles on the post-exp pt tiles, AV+rowsum with V-as-stationary
     (ones column yields softmax denominators), denominators DMA'd from PSUM
     into an [8, TQ] tile, one grouped reciprocal, K=8 selection-matrix
     matmul broadcasts the reciprocals per pair, per-pair normalize multiply.
  C) out = attnT.T @ wo^T per token tile, streamed 512KB output DMAs.
"""

import contextlib
import ctypes
import os
import sys
import types

import ml_dtypes
import numpy as np

import concourse.bass as bass
import concourse.mybir as mybir
import concourse.tile as tile


def _install_axon_ntff_shim():
    """bass_utils' trace path wants antenv.axon_hooks, which this image lacks.
    Provide it, backed by direct ctypes calls into libaxon_pjrt.so (same ABI
    the agent boot would use). Degrades to hook=None if the .so is absent."""
    try:
        import antenv.axon_hooks  # noqa: F401
        return
    except ImportError:
        pass

    _hook_holder = [None]
    so_path = "/opt/axon/libaxon_pjrt.so"
    if os.path.exists(so_path):
        try:
            lib = ctypes.CDLL(so_path)
            if hasattr(lib, "axon_start_nrt_profile"):
                lib.axon_start_nrt_profile.argtypes = [
                    ctypes.POINTER(ctypes.c_int64), ctypes.c_size_t]
                lib.axon_start_nrt_profile.restype = ctypes.c_int64
                lib.axon_stop_nrt_profile.argtypes = [ctypes.c_char_p]
                lib.axon_stop_nrt_profile.restype = ctypes.c_int64

                @contextlib.contextmanager
                def _hook(output_dir, device_ids):
                    import jax
                    jax.devices()
                    if device_ids:
                        ids = (ctypes.c_int64 * len(device_ids))(*device_ids)
                        rc = lib.axon_start_nrt_profile(ids, len(device_ids))
                    else:
                        rc = lib.axon_start_nrt_profile(None, 0)
                    if rc != 0:
                        raise RuntimeError(f"axon_start_nrt_profile rc={rc}")
                    try:
                        yield
                    finally:
                        n = lib.axon_stop_nrt_profile(str(output_dir).encode())
                        if n < 0:
                            raise RuntimeError(f"axon_stop_nrt_profile rc={n}")

                _hook_holder[0] = _hook
        except OSError:
            pass

    mod = types.ModuleType("antenv.axon_hooks")
    mod.get_axon_ntff_profile_hook = lambda: _hook_holder[0]
    mod.set_axon_ntff_profile_hook = lambda h: _hook_holder.__setitem__(0, h)
    sys.modules["antenv.axon_hooks"] = mod


_install_axon_ntff_shim()

from concourse.bass_utils import run_bass_kernel_spmd  # noqa: E402
from bass_rust import ScopedClock  # noqa: E402

B, T, DIM = 2, 2048, 1024
H, DH, WINDOW = 16, 64, 512
P = 128
TQ = 512            # queries per core
TKV = 1024          # kv tokens per core (incl. 512 halo)
NCORES = 8
NQT = TQ // P       # 4
NKT = TKV // P      # 8
NDC = DIM // P      # 8
EXP_C = 10.0        # exp offset; true max masked score is ~6.0 for this data
F32 = mybir.dt.float32
F32R = mybir.dt.float32r
BF16 = mybir.dt.bfloat16
PREC = os.environ.get("KERNEL_PREC", "bf16")
DT = F32R if PREC == "fp32r" else BF16
AF = mybir.ActivationFunctionType
ALU = mybir.AluOpType

SLOPES = [2.0 ** (-0.5 * (h + 1)) for h in range(H)]

# Score-tile packing: per head, scores are computed as S^T [key, query] in two
# 3-bank PSUM halves of [128, 1536]. Key-tile kt covers queries
# [KT_QLO[kt], KT_QLO[kt]+KT_W[kt]) at column offset KT_OFF[kt] of its half.
KT_W = [256, 256, 512, 512, 512, 512, 256, 256]
KT_OFF = [0, 256, 512, 1024, 0, 512, 1024, 1280]
KT_QLO = [0, 0, 0, 0, 0, 0, 256, 256]
HW_HALF = 1536
# post-exp boundary cleanup per half: (col0, kind) with kind 'win' (keep
# key>=query within the block), 'causal' (keep query>=key), 'dead' (zero all).
# Block (kt, qb) is valid iff qb <= kt <= qb+4; kt==qb is the window edge,
# kt==qb+4 the causal edge; anything else computed by the packing is dead.
PT_FIX = {
    0: [(0, "win"), (128, "dead"), (256 + 128, "win"),
        (512 + 256, "win"), (512 + 384, "dead"), (1024 + 384, "win")],
    1: [(0, "causal"), (512, "dead"), (512 + 128, "causal"),
        (1024, "causal"), (1280, "dead"), (1280 + 128, "causal")],
}
# merged AV matmul plan: (kt, out_col_lo, width); the four middle key tiles
# serve all 512 queries in one 512-wide matmul, edge tiles only their 256-
# query block (per-element PSUM has_written handles the region accumulation;
# the first, full-width matmul initializes every element)
AV_PLAN = [
    (2, 0, 512), (3, 0, 512), (4, 0, 512), (5, 0, 512),
    (0, 0, 256), (1, 0, 256), (6, 256, 256), (7, 256, 256),
]


class _SplitDrainTileContext(tile.TileContext):
    """The walrus build in this env rejects >1-2 sync-wait commands on one
    instruction; spread excess waits across same-engine NOPs placed directly
    before the over-limit instruction (per-engine program order preserved)."""

    def _split_excess_waits(self):
        nc = self.nc
        cur_list = nc.cur_bb.bb.instructions
        for blk in nc.m.functions[0].blocks:
            snapshot = list(blk.instructions)
            for inst in snapshot:
                si = inst.sync_info
                max_w = 1
                if si is None or len(si.on_wait) <= max_w:
                    continue
                waits = list(si.on_wait)
                si.on_wait = waits[:max_w]
                eng_obj = nc.engines[inst.engine]
                for w in waits[max_w:]:
                    nop_bi = eng_obj.nop(nofuse=True, hint="wait_split")
                    nop_inst = nop_bi.ins
                    nop_inst.sync_info = mybir.SyncInfo(on_wait=[w], on_update=[])
                    cur_list.remove(nop_inst)
                    blk.instructions.insert(
                        blk.instructions.index(inst), nop_inst)

    def _drain_and_barrier(self, tick_clock, wait_clock):
        self._split_excess_waits()
        drain_inst = self.nc.sync.drain()
        wait_clock.add_sem_waits(
            drain_inst.ins, ScopedClock({None: tick_clock.global_clock})
        )
        si = drain_inst.ins.sync_info
        if si is not None and len(si.on_wait) > 1:
            waits = list(si.on_wait)
            si.on_wait = waits[:1]
            for w in waits[1:]:
                nop = self.nc.sync.nop(nofuse=True, hint="drain_wait_split")
                nop.ins.sync_info = mybir.SyncInfo(on_wait=[w], on_update=[])
        self.nc.all_engine_barrier()
        assert self.sems is not None
        popped = self.nc._tile_sem_poison_stack.pop()
        assert popped is self._sem_poison
        self.nc.clear_and_free_semaphores(list(self.sems.allocated().values()))
        self.nc.all_engine_barrier()


def _src_nonce():
    import zlib
    with open(__file__, "rb") as f:
        return (zlib.crc32(f.read() + PREC.encode()) % 2048) + 8


def _pt_fixups(nc, pt, half):
    """Zero the invalid regions of a post-exp pt tile on the gpsimd engine."""
    for col0, kind in PT_FIX[half]:
        sl = pt[:, col0:col0 + P]
        if kind == "dead":
            nc.gpsimd.memset(sl, 0.0)
        elif kind == "win":
            # keep iff key_local >= query_local  (iota = p - qi >= 0)
            nc.gpsimd.affine_select(
                out=sl, in_=sl, compare_op=ALU.is_ge, fill=0.0,
                base=0, pattern=[[-1, P]], channel_multiplier=1)
        else:
            # causal: keep iff query_local >= key_local (iota = qi - p >= 0)
            nc.gpsimd.affine_select(
                out=sl, in_=sl, compare_op=ALU.is_ge, fill=0.0,
                base=0, pattern=[[1, P]], channel_multiplier=-1)


def _build_program(debug=False):
    nc = bass.Bass()
    # dummy input whose shape changes with this file: busts HLO-keyed NEFF
    # caches (the BIR itself is not part of the HLO fingerprint)
    nonce = nc.declare_dram_parameter("nonce", [1, _src_nonce()], F32,
                                      isOutput=False)
    xT = nc.declare_dram_parameter("xT", [DIM, TKV], DT, isOutput=False)
    # weights pre-permuted on host to [partition, slice, dc, 512] so every
    # DMA line is one contiguous 8KB-per-partition segment
    wT = nc.declare_dram_parameter("wT", [P, 6, NDC, 512], DT, isOutput=False)
    woT = nc.declare_dram_parameter("woT", [P, 2, NDC, 512], DT, isOutput=False)
    qext = nc.declare_dram_parameter("qext", [4, H, TQ], DT, isOutput=False)
    kext = nc.declare_dram_parameter("kext", [4, H, TKV], DT, isOutput=False)
    gam = nc.declare_dram_parameter("gam", [DH, 2 * H], F32, isOutput=False)
    out = nc.declare_dram_parameter("out", [TQ, DIM], F32, isOutput=True)
    if debug:
        qT_d = nc.declare_dram_parameter("qT_d", [P, NDC, TQ], DT, isOutput=True)
        kT_d = nc.declare_dram_parameter("kT_d", [P, NDC, TKV], DT, isOutput=True)
        V_d = nc.declare_dram_parameter("V_d", [P, NKT, H, DH + 1], DT, isOutput=True)
        aT_d = nc.declare_dram_parameter("aT_d", [P, NDC, TQ], DT, isOutput=True)
        pt_d = nc.declare_dram_parameter("pt_d", [2, P, 2, HW_HALF], DT, isOutput=True)

    with _SplitDrainTileContext(nc) as tc, \
            tc.tile_pool(name="persist", bufs=1) as pp, \
            tc.tile_pool(name="small", bufs=1) as psm:

        # per-head score operands: rows 0..63 = head dims, rows 64..67 = the
        # ALiBi rank-4 rows (folded into the same 68-partition contraction)
        qT = pp.tile([P, H, TQ], DT, tag="qT")         # [p, h, tok]
        kT = pp.tile([P, H, TKV], DT, tag="kT")
        V = pp.tile([P, NKT, H, DH + 1], DT, tag="V")  # [p=tok, kt, h, dh+ones]
        attnT = pp.tile([P, NDC, TQ], DT, tag="attnT")
        gam_sb = pp.tile([DH, 2 * H], F32, tag="gam")
        nc.sync.dma_start(gam_sb[:], gam[:])
        ones_sb = pp.tile([P, 1], F32R, tag="ones")
        ones_row = pp.tile([1, P], F32, tag="onesrow")
        negc_sb = pp.tile([P, 1], F32, tag="negc")
        eps_sb = pp.tile([1, 1], F32, tag="eps")
        ln8_sb = pp.tile([1, 1], F32, tag="ln8")
        # per-pair selection matrices for the reciprocal broadcast matmul:
        # sel[j][g, p] = 1 iff g == 2j + p//64   (K=8 stationary, bf16)
        sel_sb = pp.tile([8, 4, P], BF16, tag="sel")
        nc.vector.memset(ones_sb[:].bitcast(F32), 1.0)
        nc.vector.memset(ones_row[:], 1.0)
        nc.vector.memset(negc_sb[:], -EXP_C)
        nc.vector.memset(eps_sb[:], 1.0e-6)
        nc.vector.memset(ln8_sb[:], float(-0.5 * np.log(64.0)))
        nc.gpsimd.memset(sel_sb[:], 0.0)
        for j in range(4):
            nc.gpsimd.affine_select(
                out=sel_sb[:, j, :], in_=sel_sb[:, j, :],
                compare_op=ALU.not_equal, fill=1.0,
                base=-2 * j, pattern=[[-1, 2], [0, 64]], channel_multiplier=1)
        nonce_sb = pp.tile([1, _src_nonce()], F32, tag="nonce")
        nc.sync.dma_start(nonce_sb[:], nonce[:])
        nc.sync.dma_start(qT[64:68], qext[:])
        nc.sync.dma_start(kT[64:68], kext[:])
        ones_col = V[:, :, :, DH]
        nc.vector.memset(
            ones_col if DT == BF16 else ones_col.bitcast(F32), 1.0)

        # ---------------- Phase A: projections + RMSNorm ----------------
        with tc.tile_pool(name="xp", bufs=1) as px, \
                tc.tile_pool(name="wp", bufs=2) as pw, \
                tc.tile_pool(name="sqp", bufs=2) as psq, \
                tc.tile_pool(name="accp", bufs=1) as pacc, \
                tc.tile_pool(name="psA", bufs=4, space="PSUM") as psA, \
                tc.tile_pool(name="psS1", bufs=2, space="PSUM") as psS1, \
                tc.tile_pool(name="psBC", bufs=2, space="PSUM") as psBC:

            def _dma_w(dst, idx):
                # dc-pair chunks: full 128 partitions, 2KB contiguous lines
                for g in range(4):
                    nc.sync.dma_start(dst[:, 2 * g:2 * (g + 1), :],
                                      wT[:, idx, 2 * g:2 * (g + 1), :])

            x_sb = px.tile([P, NDC, TKV], DT, tag="x")
            wq0_sb = pw.tile([P, NDC, 512], DT, tag="wslice", name="wq0")
            _dma_w(wq0_sb, 0)
            for dc in range(NDC):
                nc.sync.dma_start(x_sb[:, dc, :],
                                  xT[dc * P:(dc + 1) * P, :])

            def _rms_reduce(proj, doff, acc):
                # rsqrt(mean+eps) = exp(-0.5*ln(ss/DIM + eps)); the 1/sqrt(dh)
                # score scale folds into the exp bias for q
                ss = psS1.tile([1, 512], F32, tag="ssq", name="ssq")
                nc.tensor.matmul(ss[:], ones_sb[:].bitcast(F32), acc[:],
                                 start=True, stop=True)
                a = psm.tile([1, 512], F32, tag="a")
                nc.scalar.activation(a[:], ss[:], AF.Ln,
                                     bias=eps_sb[:], scale=1.0 / DIM)
                y = psm.tile([1, 512], F32, tag="y")
                nc.scalar.activation(y[:], a[:], AF.Exp,
                                     bias=(ln8_sb[:] if proj == 0 else 0.0),
                                     scale=-0.5)
                return y

            def _rms_bcast(y):
                # broadcast over partitions via K=1 ones-matmul (plain fp32)
                bc = psBC.tile([P, 512], F32, tag="bc", name="bc")
                nc.tensor.matmul(bc[:], ones_row[:], y[:], start=True, stop=True)
                return bc

            def _normalize(proj, doffs, bcs):
                dst = qT if proj == 0 else kT
                for odt in range(NDC):
                    for hi in range(2):
                        h = 2 * odt + hi
                        gap = gam_sb[:, proj * H + h: proj * H + h + 1]
                        for doff in doffs:
                            sl = dst[0:DH, h, doff:doff + 512]
                            nc.vector.scalar_tensor_tensor(
                                sl, sl, gap, bcs[doff][0:DH, :],
                                op0=ALU.mult, op1=ALU.mult,
                            )

            # Q (tokens 512..1023 of the kv range) and K (all tokens);
            # K group (512,512) first so it only needs the priority x half
            sqacc = {}
            for proj in range(2):
                dst = qT if proj == 0 else kT
                groups = [(TKV - TQ, 0)] if proj == 0 else [(512, 512), (0, 0)]
                for wh in range(2):
                    if proj == 0 and wh == 0:
                        w_sb = wq0_sb
                    else:
                        w_sb = pw.tile([P, NDC, 512], DT, tag="wslice")
                        _dma_w(w_sb, proj * 2 + wh)
                    for ol in range(4):
                        odt = wh * 4 + ol
                        for (soff, doff) in groups:
                            ps = psA.tile([P, 512], F32, tag="projps")
                            for dc in range(NDC):
                                nc.tensor.matmul(
                                    ps[:],
                                    w_sb[:, dc, ol * P:(ol + 1) * P],
                                    x_sb[:, dc, soff:soff + 512],
                                    start=(dc == 0), stop=(dc == NDC - 1),
                                )
                            # square first: the rms chain (ACT FIFO) must
                            # not queue behind the eviction copies
                            sq = psq.tile([P, 512], F32, tag="sq")
                            nc.scalar.activation(sq[:], ps[:], AF.Square)
                            for hi in range(2):
                                nc.scalar.copy(
                                    dst[0:DH, 2 * odt + hi, doff:doff + 512],
                                    ps[DH * hi:DH * (hi + 1), :])
                            key = (proj, doff)
                            if odt == 0:
                                acc = pacc.tile([P, 512], F32,
                                                tag=f"acc{proj}_{doff}",
                                                name="acc")
                                sqacc[key] = acc
                                nc.vector.tensor_copy(acc[:], sq[:])
                            else:
                                nc.vector.tensor_add(sqacc[key][:],
                                                     sqacc[key][:], sq[:])
                if proj == 0:
                    # Q's rms chain + normalize can complete during K proj
                    y_q = _rms_reduce(0, 0, sqacc[(0, 0)])
                    _normalize(0, [0], {0: _rms_bcast(y_q)})

            # V projection: [tok, head, dh]; vh innermost so consecutive
            # matmuls share the x-chunk stationary operand; K's rms chain is
            # interleaved so its PE/ACT legs hide behind the V matmuls and
            # the scores can start right at the end of phase A
            wv_sb = []
            for vh in range(2):
                w_sb = pw.tile([P, NDC, 512], DT, tag="wslice")
                _dma_w(w_sb, 4 + vh)
                wv_sb.append(w_sb)
            y_k = {}
            for vt, tt in enumerate((4, 5, 6, 7, 0, 1, 2, 3)):
                pss = [psA.tile([P, 512], F32, tag="projps", name="psv")
                       for _ in range(2)]
                for dc in range(NDC):
                    for vh in range(2):
                        nc.tensor.matmul(
                            pss[vh][:],
                            x_sb[:, dc, tt * P:(tt + 1) * P],
                            wv_sb[vh][:, dc, :],
                            start=(dc == 0), stop=(dc == NDC - 1),
                        )
                for vh in range(2):
                    nc.scalar.copy(
                        V[:, tt, vh * 8:(vh + 1) * 8, :DH],
                        pss[vh][:].rearrange("p (h c) -> p h c", c=DH),
                    )
                if vt == 1:
                    y_k[0] = _rms_reduce(1, 0, sqacc[(1, 0)])
                    y_k[512] = _rms_reduce(1, 512, sqacc[(1, 512)])
            _normalize(1, [0, 512],
                       {d: _rms_bcast(y_k[d]) for d in (0, 512)})

        if debug:
            nc.sync.dma_start(qT_d[:], qT[:])
            nc.sync.dma_start(kT_d[:], kT[:])
            nc.sync.dma_start(V_d[:], V[:])

        # wo prefetch: issued now so the DMAs drain during phase B
        pwo = tc.alloc_tile_pool(name="wop", bufs=2)
        wo_sbs = []
        for oh in range(2):
            w_sb = pwo.tile([P, NDC, 512], DT, tag="wo")
            for g in range(4):
                nc.sync.dma_start(w_sb[:, 2 * g:2 * (g + 1), :],
                                  woT[:, oh, 2 * g:2 * (g + 1), :])
            wo_sbs.append(w_sb)

        # ---------------- Phase B: attention (head pairs) ----------------
        with tc.tile_pool(name="maskp", bufs=1) as pm, \
                tc.tile_pool(name="ptp", bufs=2) as ppt, \
                tc.tile_pool(name="rnp", bufs=1) as prn, \
                tc.tile_pool(name="psS", bufs=1, space="PSUM") as psS, \
                tc.tile_pool(name="psO", bufs=1, space="PSUM") as psO:

            s16 = [pp.tile([8, TQ], F32, tag="s16a", name="s16a"),
                   pp.tile([8, TQ], F32, tag="s16b", name="s16b")]

            for hp in range(NDC):
                ps_o = [psO.tile([DH + 1, TQ], F32, tag=f"avps{hi}", name=f"avps{hi}")
                        for hi in range(2)]
                pts = {0: [], 1: []}
                for half in range(2):
                    ps_pair = [psS.tile([P, HW_HALF], F32, tag=f"sps{hi}", name=f"sps{hi}")
                               for hi in range(2)]
                    for ktl in range(4):
                        kt = half * 4 + ktl
                        off, wdt, qlo = KT_OFF[kt], KT_W[kt], KT_QLO[kt]
                        for hi in range(2):
                            h = 2 * hp + hi
                            # 68-row contraction: head dims + ALiBi rank-4
                            nc.tensor.matmul(
                                ps_pair[hi][:, off:off + wdt],
                                kT[0:DH + 4, h, kt * P:(kt + 1) * P],
                                qT[0:DH + 4, h, qlo:qlo + wdt],
                                start=True, stop=True,
                            )
                    for hi in range(2):
                        pt = ppt.tile([P, HW_HALF], DT, tag=f"pt{hi}")
                        nc.scalar.activation(pt[:], ps_pair[hi][:], AF.Exp,
                                             bias=negc_sb[:])
                        _pt_fixups(nc, pt, half)
                        if debug and hp == 0:
                            nc.sync.dma_start(pt_d[hi, :, half, :], pt[:])
                        pts[hi].append(pt)

                for hi in range(2):
                    h = 2 * hp + hi
                    po = DH * hi
                    # middle key tiles cover all 512 queries in one matmul;
                    # edge tiles touch only their 256-query block
                    for i, (kt, q0, w) in enumerate(AV_PLAN):
                        c0 = KT_OFF[kt] + q0 - KT_QLO[kt]
                        nc.tensor.matmul(
                            ps_o[hi][:, q0:q0 + w],
                            V[:, kt, h, :],
                            pts[hi][kt // 4][:, c0:c0 + w],
                            start=(i == 0), stop=(i == len(AV_PLAN) - 1),
                            skip_group_check=True,
                        )
                    # stash the softmax denominator row; head h -> partition
                    # h of s16 via a tiny SBUF->SBUF DMA (engines cannot write
                    # partition offsets other than 0/32/64)
                    r = prn.tile([1, TQ], F32, tag=f"r{hi}", name="r")
                    nc.vector.tensor_copy(r[:], ps_o[hi][DH:DH + 1, :])
                    nc.sync.dma_start(s16[h // 8][h % 8:h % 8 + 1, :], r[:])
                    nc.vector.tensor_copy(attnT[po:po + DH, hp, :], ps_o[hi][:DH, :])

                if hp == 3:
                    # normalize the finished half (8 heads): batched reciprocal
                    # on 8 partitions, K=8 selection-matrix matmul broadcasts
                    # each pair's rows across its 128 partitions; group 1's
                    # normalize is deferred into phase C so its reciprocal
                    # chain hides behind the first output-projection matmuls
                    rc8 = pm.tile([8, TQ], F32, tag="rc0", name="rc8")
                    nc.vector.reciprocal(rc8[:], s16[0][:])
                    rc8b = pm.tile([8, TQ], BF16, tag="rcb0", name="rc8b")
                    nc.vector.tensor_copy(rc8b[:], rc8[:])
                    for hp2 in range(4):
                        rb = psO.tile([P, TQ], F32,
                                      tag=f"avps{hp2 % 2}", name="rb")
                        nc.tensor.matmul(
                            rb[:], sel_sb[:, hp2, :], rc8b[:],
                            start=True, stop=True)
                        nc.vector.tensor_mul(attnT[:, hp2, :],
                                             attnT[:, hp2, :], rb[:])

        if debug:
            nc.sync.dma_start(aT_d[:], attnT[:])

        # ---------------- Phase C: output projection ----------------
        with tc.tile_pool(name="outp", bufs=1) as pout, \
                tc.tile_pool(name="psC", bufs=1, space="PSUM") as psC:
            out_sb = pout.tile([P, NQT, DIM], F32, tag="out")
            out_r = out.rearrange("(tt p) o -> p tt o", p=P)

            def _c_mms(pss, tt, adcs):
                for adc in adcs:
                    for oh in range(2):
                        nc.tensor.matmul(
                            pss[oh][:],
                            attnT[:, adc, tt * P:(tt + 1) * P],
                            wo_sbs[oh][:, adc, :],
                            start=(adc == 0), stop=(adc == NDC - 1),
                            skip_group_check=True,
                        )

            def _c_finish(pss, tt):
                for oh in range(2):
                    nc.vector.tensor_copy(
                        out_sb[:, tt, oh * 512:(oh + 1) * 512], pss[oh][:])
                for g in range(4):
                    nc.sync.dma_start(
                        out_r[32 * g:32 * (g + 1), tt, :],
                        out_sb[32 * g:32 * (g + 1), tt, :])

            # tt0/tt1 over the already-normalized first 8 heads, overlapping
            # group 1's reciprocal/broadcast chain
            psfront = {}
            for tt in range(2):
                psfront[tt] = [psC.tile([P, 512], F32, tag=f"cps{tt}_{oh}",
                                        name="psc") for oh in range(2)]
                _c_mms(psfront[tt], tt, range(4))

            rc8 = pout.tile([8, TQ], F32, tag="rc1", name="rc8")
            nc.vector.reciprocal(rc8[:], s16[1][:])
            rc8b = pout.tile([8, TQ], BF16, tag="rcb1", name="rc8b")
            nc.vector.tensor_copy(rc8b[:], rc8[:])
            for hp2 in range(4, 8):
                rb = psC.tile([P, TQ], F32, tag="rb", name="rb")
                nc.tensor.matmul(
                    rb[:], sel_sb[:, hp2 % 4, :], rc8b[:],
                    start=True, stop=True)
                nc.vector.tensor_mul(attnT[:, hp2, :],
                                     attnT[:, hp2, :], rb[:])

            for tt in range(2):
                _c_mms(psfront[tt], tt, range(4, NDC))
                _c_finish(psfront[tt], tt)
            for tt in range(2, NQT):
                pss = [psC.tile([P, 512], F32, tag=f"cps{tt % 2}_{oh}",
                                name="psc") for oh in range(2)]
                _c_mms(pss, tt, range(NDC))
                _c_finish(pss, tt)
        pwo.release()

    return nc


def _np_dt():
    return np.float32 if PREC == "fp32r" else ml_dtypes.bfloat16


def _build_ext(chunk0: bool):
    """qext [4,H,TQ]: (hi_q, lo_q, 1, 1);  kext [4,H,TKV]: (1, 1, k_hi, k_lo).
    hi/lo are a two-term bf16 split of -slope_h*(q_local+512); k_hi/k_lo of
    slope_h*k_local. Padded halo keys (chunk 0) get k_hi = -3e4 -> exp = 0."""
    np_dt = _np_dt()
    qe = np.zeros((4, H, TQ), np.float32)
    ke = np.zeros((4, H, TKV), np.float32)
    qpos = np.arange(TQ, dtype=np.float64) + 512.0
    kpos = np.arange(TKV, dtype=np.float64)
    qe[2:] = 1.0
    ke[:2] = 1.0
    qhi = np.zeros((H, TQ), np_dt)
    qlo = np.zeros((H, TQ), np_dt)
    khi = np.zeros((H, TKV), np_dt)
    klo = np.zeros((H, TKV), np_dt)
    for h in range(H):
        tq = (-SLOPES[h] * qpos).astype(np.float32)
        hi = tq.astype(np_dt)
        qhi[h] = hi
        qlo[h] = (tq - hi.astype(np.float32)).astype(np_dt)
        tk = (SLOPES[h] * kpos).astype(np.float32)
        if chunk0:
            tk[:512] = -3.0e4
        hi = tk.astype(np_dt)
        khi[h] = hi
        lo = (tk - hi.astype(np.float32))
        if chunk0:
            lo[:512] = 0.0
        klo[h] = lo.astype(np_dt)
    qe = qe.astype(np_dt)
    ke = ke.astype(np_dt)
    qe[0], qe[1] = qhi, qlo
    ke[2], ke[3] = khi, klo
    return np.ascontiguousarray(qe), np.ascontiguousarray(ke)


_NC = None
LAST = None  # BassKernelResults of the most recent run (exec_time_ns when traced)


def _get_nc():
    global _NC
    if _NC is None:
        _NC = _build_program()
    return _NC


def kernel(x, wq, wk, wv, wo, q_gamma, k_gamma):
    x = np.ascontiguousarray(np.asarray(x, np.float32))
    wq = np.asarray(wq, np.float32)
    wk = np.asarray(wk, np.float32)
    wv = np.asarray(wv, np.float32)
    wo = np.asarray(wo, np.float32)
    q_gamma = np.asarray(q_gamma, np.float32)
    k_gamma = np.asarray(k_gamma, np.float32)

    np_dt = _np_dt()
    # [in, out] -> [p, slice, dc, o]: per-partition-contiguous 8KB DMA lines
    wcat = np.concatenate([wq.T, wk.T, wv.T], axis=1)
    wT_host = np.ascontiguousarray(
        wcat.reshape(NDC, P, 6, 512).transpose(1, 2, 0, 3).astype(np_dt))
    woT_host = np.ascontiguousarray(
        wo.T.reshape(NDC, P, 2, 512).transpose(1, 2, 0, 3).astype(np_dt))
    gam_host = np.ascontiguousarray(np.concatenate(
        [q_gamma.reshape(H, DH).T, k_gamma.reshape(H, DH).T], axis=1))
    qe0, ke0 = _build_ext(True)
    qei, kei = _build_ext(False)

    in_maps = []
    for c in range(NCORES):
        b, j = divmod(c, 4)
        lo = j * TQ - WINDOW
        xs = x[b, max(0, lo): j * TQ + TQ, :]
        if lo < 0:
            xs = np.concatenate(
                [np.zeros((-lo, DIM), np.float32), xs], axis=0)
        in_maps.append({
            "nonce": np.zeros((1, _src_nonce()), np.float32),
            "xT": np.ascontiguousarray(xs.T.astype(np_dt)),
            "wT": wT_host,
            "woT": woT_host,
            "gam": gam_host,
            "qext": qe0 if j == 0 else qei,
            "kext": ke0 if j == 0 else kei,
        })

    global LAST
    trace = bool(int(os.environ.get("KERNEL_TRACE", "0") or 0))
    try:
        LAST = run_bass_kernel_spmd(
            _get_nc(), in_maps, list(range(NCORES)), trace=trace)
    except Exception:
        # a previously-wedged device surfaces as NRT_EXEC_UNIT_UNRECOVERABLE
        # on the first touch; reset the accelerator once and retry
        try:
            lib = ctypes.CDLL("/opt/axon/libaxon_pjrt.so")
            lib.axon_reset.restype = ctypes.c_int64
            import jax
            jax.devices()
            lib.axon_reset()
        except Exception:
            pass
        LAST = run_bass_kernel_spmd(
            _get_nc(), in_maps, list(range(NCORES)), trace=trace)

    full = np.empty((B, T, DIM), np.float32)
    for c in range(NCORES):
        b, j = divmod(c, 4)
        full[b, j * TQ:(j + 1) * TQ, :] = LAST.results[c]["out"]
    return full
